# revision 10
# baseline (speedup 1.0000x reference)
"""Trainium2 Bass kernel for nn_AttentionBlock (B=2,S=2048,D=1024,H=16,MLP=4096).

Key structural insight: the reference does q.reshape(B,H,S,HD) on a row-major
[B,S,D] tensor (no transpose), so head hh consumes ROWS 128*hh:128*(hh+1) of
the projected [2048,1024] matrix reinterpreted as [2048,64]. The entire block
(LN1 -> QKV -> attention -> residual -> LN2 -> MLP -> residual) therefore
decomposes into 32 independent 128-row blocks (B*H), aligned across all ops.
Sharding: 8 cores x 4 head-blocks (512 rows) each, zero collectives.

Per-core device program (all matmuls bf16, accum f32):
  - LN1 (bn_stats) -> transpose via PE -> hT [1024,512] (per-partition affine)
  - qprojT/kprojT matmuls; rearranged into per-head [64, c, 128] blocks
    (qblk/kblk); vproj natural -> vaug with interleaved ones columns
  - scoresT per (head, e-block): 16 K=64 matmuls from qblk/kblk sub-blocks,
    + host-permuted mask add (DVE), exp (ACT, no max-subtraction: scores
    are bounded), then accumulate out^T = [V|1]^T @ expT into PSUM, which
    yields softmax numerator AND denominator in one accumulation.
  - transpose-back per 128-col block + normalize -> h_attn
  - residual + LN2 -> h2T; MLP: mlp1T = w1-chunks @ h2T, exact Gelu (ACT),
    mlp2 natural with ghT chunks as lhsT; final residual add, DMA out.
"""
import sys

sys.path.insert(0, "/opt/trn_rl_repo")

import numpy as np
import ml_dtypes

import concourse.bass as bass
import concourse.mybir as mybir
import concourse.tile as tile
from concourse.bass_utils import run_bass_kernel_spmd
from concourse.masks import make_identity

B, S, D = 2, 2048, 1024
H, HD, MLP = 16, 64, 4096
EPS = 1e-5
R = 512          # rows per core
NCORES = 8
F32 = mybir.dt.float32
BF16 = mybir.dt.bfloat16
BF = ml_dtypes.bfloat16
AF = mybir.ActivationFunctionType
AOP = mybir.AluOpType


# walrus in this container rejects >1 sync-wait on TPB_CTRL (Drain): split the
# TileContext final-drain waits across sequential drains (same AND semantics).
def _patch_drain():
    if getattr(tile.TileContext, "_dab_patched", False):
        return

    def _patched_dab(self, tick_clock, wait_clock):
        from concourse.vector_clock import ScopedClock
        drain_inst = self.nc.sync.drain()
        wait_clock.add_sem_waits(drain_inst.ins,
                                 ScopedClock({None: tick_clock.global_clock}))
        si = drain_inst.ins.sync_info
        if si is not None and len(si.on_wait) > 1:
            waits = list(si.on_wait)
            drain_inst.ins.sync_info = mybir.SyncInfo(on_wait=waits[:1],
                                                      on_update=list(si.on_update))
            for w in waits[1:]:
                extra = self.nc.sync.drain()
                extra.ins.sync_info = mybir.SyncInfo(on_wait=[w], on_update=[])
        self.nc.all_engine_barrier()
        assert self.sems is not None
        popped = self.nc._tile_sem_poison_stack.pop()
        assert popped is self._sem_poison
        self.nc.clear_and_free_semaphores(list(self.sems.allocated().values()))
        self.nc.all_engine_barrier()

    tile.TileContext._drain_and_barrier = _patched_dab
    tile.TileContext._dab_patched = True


# This walrus build accepts at most ONE sync-wait per instruction
# (setupSyncWait raises "Too many sync wait commands" otherwise).  Hoist
# excess waits onto same-engine NoOp carriers placed immediately before the
# instruction: engine streams execute in order, so waiting on the carrier
# then the instruction is equivalent to the instruction waiting on all.
_WAIT_LIMIT = 1


def _split_waits(nc):
    n_carriers = 0
    for bbname, bbw in nc.bb_map.items():
        il = bbw.bb.instructions
        out = []
        for inst in il:
            si = inst.sync_info
            if si is not None and len(si.on_wait) > _WAIT_LIMIT:
                waits = list(si.on_wait)
                extra, keep = waits[:-_WAIT_LIMIT], waits[-_WAIT_LIMIT:]
                for w in extra:
                    nop = mybir.InstNoOp(name=f"wsplit_{n_carriers}", ins=[], outs=[])
                    nop.engine = inst.engine
                    nop.sync_info = mybir.SyncInfo(on_wait=[w], on_update=[])
                    nc.register_instruction(nop, overwrite=True)
                    out.append(nop)
                    n_carriers += 1
                inst.sync_info = mybir.SyncInfo(on_wait=keep,
                                                on_update=list(si.on_update))
            out.append(inst)
        bbw.bb.instructions = out
    return n_carriers


def _phase_a(nc, tc, pp, pAB, consts, dram):
    """LN1 -> hT, QKV projections -> qblk/kblk/vaug. Returns nothing."""
    ident, eps_t, cols = consts["ident_bf"], consts["eps"], consts
    x_sb = consts["x_sb"]
    qblk, kblk, vaug, mask_sb = (consts["qblk"], consts["kblk"],
                                 consts["vaug"], consts["mask_sb"])
    with tc.tile_pool(name="phA", bufs=1) as pA, \
         tc.tile_pool(name="stat", bufs=8) as stat, \
         tc.tile_pool(name="psP", bufs=4, space="PSUM") as psP, \
         tc.tile_pool(name="wload", bufs=1) as wl, \
         tc.tile_pool(name="stg", bufs=2) as stg:
        xc = pA.tile([128, 4, D], BF16, tag="xc")
        hT = pA.tile([128, 8, R], BF16, tag="hT")
        x_r = dram["x"].rearrange("(n p) d -> n p d", p=128)
        for i in range(4):
            nc.sync.dma_start(x_sb[:, i, :], x_r[i])
            st = stat.tile([128, 2, 6], F32, tag="st")
            nc.vector.bn_stats(st[:, 0, :], x_sb[:, i, 0:512])
            nc.vector.bn_stats(st[:, 1, :], x_sb[:, i, 512:D])
            mv = stat.tile([128, 2], F32, tag="mv")
            nc.vector.bn_aggr(mv, st)
            sd = stat.tile([128, 1], F32, tag="sd")
            nc.scalar.activation(sd, mv[:, 1:2], AF.Sqrt, bias=eps_t, scale=1.0)
            rs = stat.tile([128, 1], F32, tag="rs")
            nc.vector.reciprocal(rs, sd)
            nc.vector.tensor_scalar(out=xc[:, i, :], in0=x_sb[:, i, :],
                                    scalar1=mv[:, 0:1], scalar2=rs,
                                    op0=AOP.subtract, op1=AOP.mult)
        for dt8 in range(8):
            for rt in range(4):
                nc.sync.dma_start(hT[:, dt8, 128 * rt:128 * rt + 128],
                                  xc[:, rt, 128 * dt8:128 * dt8 + 128],
                                  transpose=True)
            nc.vector.tensor_scalar(out=hT[:, dt8, :], in0=hT[:, dt8, :],
                                    scalar1=cols["g1"][:, dt8:dt8 + 1],
                                    scalar2=cols["b1"][:, dt8:dt8 + 1],
                                    op0=AOP.mult, op1=AOP.add)
        # Q and K projections (transposed layout) -> qblk/kblk
        for wname, bname, blk in (("wq", "bq", qblk), ("wk", "bk", kblk)):
            w_sb = wl.tile([128, 8, D], BF16, tag="w", name=f"w_{wname}")
            nc.sync.dma_start(w_sb, dram[wname].rearrange("(c p) d -> p c d", p=128))
            b_c = cols[bname]
            for t8 in range(8):
                ps = psP.tile([128, R], F32, tag="pp", name=f"ps_{wname}_{t8}")
                for ci in range(8):
                    nc.tensor.matmul(ps, w_sb[:, ci, 128 * t8:128 * t8 + 128],
                                     hT[:, ci, :], start=(ci == 0), stop=(ci == 7))
                sg = stg.tile([128, R], BF16, tag="sg", name=f"sg_{wname}_{t8}")
                nc.vector.tensor_scalar(out=sg, in0=ps, scalar1=b_c[:, t8:t8 + 1],
                                        scalar2=None, op0=AOP.add)
                nc.sync.dma_start(blk[:, :, 2 * t8, :],
                                  sg[0:64, :].rearrange("p (j r) -> p j r", j=4))
                nc.sync.dma_start(blk[:, :, 2 * t8 + 1, :],
                                  sg[64:128, :].rearrange("p (j r) -> p j r", j=4))
        # V projection (natural layout) -> vaug
        wv_sb = wl.tile([128, 8, D], BF16, tag="w")
        nc.sync.dma_start(wv_sb, dram["wv"].rearrange("(c p) d -> p c d", p=128))
        for rt in range(4):
            for hf in range(2):
                ps = psP.tile([128, R], F32, tag="pp", name=f"ps_v_{rt}_{hf}")
                for ci in range(8):
                    nc.tensor.matmul(ps, hT[:, ci, 128 * rt:128 * rt + 128],
                                     wv_sb[:, ci, 512 * hf:512 * hf + 512],
                                     start=(ci == 0), stop=(ci == 7))
                nc.vector.tensor_copy(vaug[:, rt, 8 * hf:8 * hf + 8, 0:64],
                                      ps.rearrange("p (e dd) -> p e dd", dd=64))


def _phase_b(nc, tc, consts, hattn):
    """Attention: scoresT -> mask+exp -> V_aug accumulation -> normalize."""
    ident = consts["ident"]
    qblk, kblk, vaug, mask_sb = (consts["qblk"], consts["kblk"],
                                 consts["vaug"], consts["mask_sb"])
    with tc.tile_pool(name="ex", bufs=2) as exp_pool, \
         tc.tile_pool(name="nmsb", bufs=2) as nms, \
         tc.tile_pool(name="scps", bufs=2, space="PSUM") as scps, \
         tc.tile_pool(name="outps", bufs=1, space="PSUM") as outps, \
         tc.tile_pool(name="attst", bufs=8) as attst:
        for j in range(4):
            outp = outps.tile([65, S], F32, tag="op", name=f"op_{j}")
            for e in range(16):
                ex = exp_pool.tile([128, S], BF16, tag="ex", name=f"ex_{j}_{e}")
                exr = exp_pool.tile([128, S], BF16, tag="exr", name=f"exr_{j}_{e}")
                for hf in range(2):
                    sc = scps.tile([128, 1024], F32, tag="sc",
                                   name=f"sc_{j}_{e}_{hf}")
                    for q2 in range(2):
                        col = 512 * q2
                        c0 = 8 * hf + 4 * q2
                        nc.tensor.matmul(sc[:, col:col + 512],
                                         kblk[:, j, e, :],
                                         qblk[:, j, c0:c0 + 4, :],
                                         start=True, stop=True)
                    nc.scalar.activation(exr[:, 1024 * hf:1024 * hf + 1024], sc, AF.Exp)
                    nc.gpsimd.tensor_tensor(
                        out=ex[:, 1024 * hf:1024 * hf + 1024],
                        in0=exr[:, 1024 * hf:1024 * hf + 1024],
                        in1=mask_sb[:, e, 1024 * hf:1024 * hf + 1024],
                        op=AOP.mult)
                    for q2 in range(2):
                        col = 1024 * hf + 512 * q2
                        nc.tensor.matmul(outp[:, col:col + 512], vaug[:, j, e, :],
                                         ex[:, col:col + 512],
                                         start=(e == 0), stop=(e == 15))
            nm = nms.tile([65, S], F32, tag="nm", name=f"nm_{j}")
            nc.vector.tensor_copy(nm, outp)
            for c16 in range(16):
                tp = scps.tile([128, 1024], F32, tag="sc", name=f"tpb_{j}_{c16}")
                nc.tensor.transpose(tp[:, 0:65], nm[:, 128 * c16:128 * c16 + 128],
                                    ident[0:65, 0:65])
                rd = attst.tile([128, 1], F32, tag="rd", name=f"rd_{j}_{c16}")
                nc.vector.reciprocal(rd, tp[:, 64:65])
                nc.vector.tensor_scalar(out=hattn[:, j, 64 * c16:64 * c16 + 64],
                                        in0=tp[:, 0:64], scalar1=rd, scalar2=None,
                                        op0=AOP.mult)


def _phase_cd(nc, tc, consts, dram, hattn):
    """Residual + LN2 -> h2T; MLP; final residual; DMA out."""
    ident, eps_t = consts["ident"], consts["eps"]
    x_sb = consts["x_sb"]
    with tc.tile_pool(name="phC", bufs=1) as pC, \
         tc.tile_pool(name="stat2", bufs=8) as stat2, \
         tc.tile_pool(name="tmp2", bufs=2) as tmp2:
        xc2 = pC.tile([128, 4, D], F32, tag="xc2")
        h2T = pC.tile([128, 8, R], BF16, tag="h2T")
        for i in range(4):
            h2p = tmp2.tile([128, D], F32, tag="h2p", name=f"h2p_{i}")
            nc.vector.tensor_tensor(out=h2p, in0=hattn[:, i, :],
                                    in1=x_sb[:, i, :], op=AOP.add)
            nc.vector.tensor_tensor(out=h2p, in0=h2p, in1=consts["bv_b"], op=AOP.add)
            st = stat2.tile([128, 2, 6], F32, tag="st", name=f"st2_{i}")
            nc.vector.bn_stats(st[:, 0, :], h2p[:, 0:512])
            nc.vector.bn_stats(st[:, 1, :], h2p[:, 512:D])
            mv = stat2.tile([128, 2], F32, tag="mv", name=f"mv2_{i}")
            nc.vector.bn_aggr(mv, st)
            sd = stat2.tile([128, 1], F32, tag="sd", name=f"sd2_{i}")
            nc.scalar.activation(sd, mv[:, 1:2], AF.Sqrt, bias=eps_t, scale=1.0)
            rs = stat2.tile([128, 1], F32, tag="rs", name=f"rs2_{i}")
            nc.vector.reciprocal(rs, sd)
            nc.vector.tensor_scalar(out=xc2[:, i, :], in0=h2p,
                                    scalar1=mv[:, 0:1], scalar2=rs,
                                    op0=AOP.subtract, op1=AOP.mult)
        xc2b = pC.tile([128, 4, D], BF16, tag="xc2b")
        for rt in range(4):
            nc.vector.tensor_copy(xc2b[:, rt, :], xc2[:, rt, :])
        for dt8 in range(8):
            for rt in range(4):
                nc.sync.dma_start(h2T[:, dt8, 128 * rt:128 * rt + 128],
                                  xc2b[:, rt, 128 * dt8:128 * dt8 + 128],
                                  transpose=True)
            nc.vector.tensor_scalar(out=h2T[:, dt8, :], in0=h2T[:, dt8, :],
                                    scalar1=consts["g2"][:, dt8:dt8 + 1],
                                    scalar2=consts["b2l"][:, dt8:dt8 + 1],
                                    op0=AOP.mult, op1=AOP.add)
        ghT = pC.tile([128, 32, R], BF16, tag="ghT")
        with tc.tile_pool(name="w1l", bufs=3) as w1l, \
             tc.tile_pool(name="psD1", bufs=4, space="PSUM") as psD1:
            for mt in range(32):
                w1t = w1l.tile([128, 8, 128], BF16, tag="w1", name=f"w1t_{mt}")
                nc.sync.dma_start(w1t, dram["w1"][mt])
                ps = psD1.tile([128, R], F32, tag="m1", name=f"m1_{mt}")
                for ci in range(8):
                    nc.tensor.matmul(ps, w1t[:, ci, :], h2T[:, ci, :],
                                     start=(ci == 0), stop=(ci == 7))
                nc.scalar.activation(ghT[:, mt, :], ps, AF.Gelu,
                                     bias=consts["b1m"][:, mt:mt + 1], scale=1.0)
        resf = pC.tile([128, 4, D], F32, tag="resf")
        for rt in range(4):
            nc.vector.tensor_tensor(out=resf[:, rt, :], in0=xc2[:, rt, :],
                                    in1=consts["g2_b"], op=AOP.mult)
            nc.vector.tensor_tensor(out=resf[:, rt, :], in0=resf[:, rt, :],
                                    in1=consts["bcomb_b"], op=AOP.add)
        ostg = pC.tile([128, 4, D], F32, tag="ostg")
        out_r = dram["out"].rearrange("(n p) d -> n p d", p=128)
        with tc.tile_pool(name="w2l", bufs=3) as w2l, \
             tc.tile_pool(name="psD2", bufs=1, space="PSUM") as psD2:
            m2ps = [[psD2.tile([128, 512], F32, tag=f"m2_{rt}_{dc}",
                               name=f"m2_{rt}_{dc}")
                     for dc in range(2)] for rt in range(4)]
            for mc in range(32):
                w2t = w2l.tile([128, D], BF16, tag="w2", name=f"w2t_{mc}")
                nc.sync.dma_start(w2t, dram["w2"][128 * mc:128 * mc + 128, :])
                for rt in range(4):
                    for dc in range(2):
                        nc.tensor.matmul(m2ps[rt][dc],
                                         ghT[:, mc, 128 * rt:128 * rt + 128],
                                         w2t[:, 512 * dc:512 * dc + 512],
                                         start=(mc == 0), stop=(mc == 31))
            for rt in range(4):
                for dc in range(2):
                    nc.vector.tensor_tensor(out=ostg[:, rt, 512 * dc:512 * dc + 512],
                                            in0=m2ps[rt][dc],
                                            in1=resf[:, rt, 512 * dc:512 * dc + 512],
                                            op=AOP.add)
                nc.sync.dma_start(out_r[rt], ostg[:, rt, :])


def _build_program():
    _patch_drain()
    nc = bass.Bass()
    dram = {
        "x": nc.dram_tensor("x", [R, D], F32, kind="ExternalInput"),
        "maskp": nc.dram_tensor("maskp", [S, S], BF16, kind="ExternalInput"),
        "wq": nc.dram_tensor("wq", [D, D], BF16, kind="ExternalInput"),
        "wk": nc.dram_tensor("wk", [D, D], BF16, kind="ExternalInput"),
        "wv": nc.dram_tensor("wv", [D, D], BF16, kind="ExternalInput"),
        "bq": nc.dram_tensor("bq", [D], F32, kind="ExternalInput"),
        "bk": nc.dram_tensor("bk", [D], F32, kind="ExternalInput"),
        "bv": nc.dram_tensor("bv", [D], F32, kind="ExternalInput"),
        "g1": nc.dram_tensor("g1", [D], F32, kind="ExternalInput"),
        "b1": nc.dram_tensor("b1", [D], F32, kind="ExternalInput"),
        "g2": nc.dram_tensor("g2", [D], F32, kind="ExternalInput"),
        "b2l": nc.dram_tensor("b2l", [D], F32, kind="ExternalInput"),
        "bcomb": nc.dram_tensor("bcomb", [D], F32, kind="ExternalInput"),
        "w1": nc.dram_tensor("w1", [32, 128, 8, 128], BF16, kind="ExternalInput"),
        "b1m": nc.dram_tensor("b1m", [MLP], F32, kind="ExternalInput"),
        "w2": nc.dram_tensor("w2", [MLP, D], BF16, kind="ExternalInput"),
        "out": nc.dram_tensor("out", [R, D], F32, kind="ExternalOutput"),
    }
    with tile.TileContext(nc) as tc:
        with tc.tile_pool(name="persist", bufs=1) as pp:
            consts = {}
            ident = pp.tile([128, 128], F32, tag="ident")
            make_identity(nc, ident)
            consts["ident"] = ident
            ident_bf = pp.tile([128, 128], BF16, tag="identbf")
            make_identity(nc, ident_bf)
            consts["ident_bf"] = ident_bf
            eps_t = pp.tile([128, 1], F32, tag="eps")
            nc.vector.memset(eps_t, EPS)
            consts["eps"] = eps_t

            def pcol(key, n):
                t_ = pp.tile([128, n], F32, tag=f"c_{key}", name=f"c_{key}")
                nc.sync.dma_start(t_, dram[key].rearrange("(t p) -> p t", p=128))
                return t_

            for key, n in (("bq", 8), ("bk", 8), ("g1", 8), ("b1", 8),
                           ("g2", 8), ("b2l", 8), ("b1m", 32)):
                consts[key] = pcol(key, n)

            def bcast(key):
                t_ = pp.tile([128, D], F32, tag=f"b_{key}", name=f"b_{key}")
                src = dram[key][:]
                bc = bass.AP(tensor=src.tensor, offset=src.offset,
                             ap=[[0, 128]] + [list(a) for a in src.ap])
                nc.sync.dma_start(t_, bc)
                return t_

            consts["bv_b"] = bcast("bv")
            consts["g2_b"] = bcast("g2")
            consts["bcomb_b"] = bcast("bcomb")

            consts["x_sb"] = pp.tile([128, 4, D], F32, tag="x", name="x_sb")

            with tc.tile_pool(name="poolBC", bufs=1) as pBC:
                with tc.tile_pool(name="poolAB", bufs=1) as pAB:
                    consts["qblk"] = pAB.tile([64, 4, 16, 128], BF16, tag="qblk",
                                              name="qblk")   # [d, j, c, rr]
                    consts["kblk"] = pAB.tile([64, 4, 16, 128], BF16, tag="kblk",
                                              name="kblk")
                    consts["vaug"] = pAB.tile([128, 4, 16, 65], BF16, tag="vaug",
                                              name="vaug")   # [rr, j, e, d|1]
                    nc.vector.memset(consts["vaug"][:, :, :, 64:65], 1.0)
                    consts["mask_sb"] = pAB.tile([128, 16, S], BF16, tag="mask",
                                                 name="mask_sb")
                    mask_r = dram["maskp"].rearrange("(e p) s -> e p s", p=128)
                    for e in range(16):
                        nc.sync.dma_start(consts["mask_sb"][:, e, :], mask_r[e])

                    _phase_a(nc, tc, pp, pAB, consts, dram)
                    hattn = pBC.tile([128, 4, D], F32, tag="hattn", name="hattn")
                    _phase_b(nc, tc, consts, hattn)
                _phase_cd(nc, tc, consts, dram, hattn)
    n = _split_waits(nc)
    print(f"[kernel] split {n} excess sync-waits onto NoOp carriers")
    return nc


_PROGRAM = None


def _get_program():
    global _PROGRAM
    if _PROGRAM is None:
        _PROGRAM = _build_program()
    return _PROGRAM


def _prep_inputs(x, dis_attn_mask, cls_attn_mask, wq, bq, wk, bk, wv, bv,
                 ln1_g, ln1_b, ln2_g, ln2_b, w1, b1, w2, b2):
    """Host-side prep: per-core shards + weight dtype/layout conversion."""
    x = np.asarray(x, np.float32)
    wq8 = (np.asarray(wq, np.float32) / 8.0).astype(BF)
    bq8 = (np.asarray(bq, np.float32) / 8.0)
    wk_b = np.asarray(wk, np.float32).astype(BF)
    wv_b = np.asarray(wv, np.float32).astype(BF)
    # w1 pre-tiled to [32(mt), 128(p=din%128), 8(c=din//128), 128(n=mout%128)]
    w1_t = np.ascontiguousarray(
        np.asarray(w1, np.float32).astype(BF).reshape(8, 128, 32, 128).transpose(2, 1, 0, 3))
    w2_b = np.asarray(w2, np.float32).astype(BF)
    bcomb = (np.asarray(ln2_b, np.float32) + np.asarray(b2, np.float32))
    masks = {}
    for bb in range(B):
        msum = (np.asarray(dis_attn_mask[bb], np.float32)
                + np.asarray(cls_attn_mask[bb], np.float32))
        masks[bb] = np.exp(np.ascontiguousarray(
            msum.reshape(128, 16, 128, 16).transpose(3, 2, 1, 0).reshape(S, S))).astype(BF)
    shared = {
        "wq": wq8, "wk": wk_b, "wv": wv_b,
        "bq": bq8.astype(np.float32), "bk": np.asarray(bk, np.float32),
        "bv": np.asarray(bv, np.float32),
        "g1": np.asarray(ln1_g, np.float32), "b1": np.asarray(ln1_b, np.float32),
        "g2": np.asarray(ln2_g, np.float32), "b2l": np.asarray(ln2_b, np.float32),
        "bcomb": bcomb.astype(np.float32),
        "w1": w1_t, "b1m": np.asarray(b1, np.float32), "w2": w2_b,
    }
    in_maps = []
    for core in range(NCORES):
        bb = core // 4
        g = core % 4
        m = dict(shared)
        m["x"] = np.ascontiguousarray(x[bb, 512 * g:512 * g + 512])
        m["maskp"] = masks[bb]
        in_maps.append(m)
    return in_maps


def kernel(**inputs):
    nc = _get_program()
    in_maps = _prep_inputs(**inputs)
    res = run_bass_kernel_spmd(nc, in_maps, core_ids=list(range(NCORES)))
    out = np.zeros((B, S, D), np.float32)
    for core in range(NCORES):
        bb = core // 4
        g = core % 4
        out[bb, 512 * g:512 * g + 512] = res.results[core]["out"]
    return out


if __name__ == "__main__":
    sys.path.insert(0, "/root/problem")
    import reference
    inputs = {k: np.asarray(v) for k, v in reference.setup_inputs().items()}
    expected = np.asarray(reference.reference(**inputs))
    actual = kernel(**inputs)
    err = np.abs(actual - expected)
    scale = np.abs(expected).max()
    print("max abs err:", err.max(), "scale:", scale, "rel:", err.max() / scale)


# revision 11
# speedup vs baseline: 1.0832x; 1.0832x over previous
"""Trainium2 Bass kernel for nn_AttentionBlock (B=2,S=2048,D=1024,H=16,MLP=4096).

Key structural insight: the reference does q.reshape(B,H,S,HD) on a row-major
[B,S,D] tensor (no transpose), so head hh consumes ROWS 128*hh:128*(hh+1) of
the projected [2048,1024] matrix reinterpreted as [2048,64]. The entire block
(LN1 -> QKV -> attention -> residual -> LN2 -> MLP -> residual) therefore
decomposes into 32 independent 128-row blocks (B*H), aligned across all ops.
Sharding: 8 cores x 4 head-blocks (512 rows) each, zero collectives.

Per-core device program (all matmuls bf16, accum f32):
  - LN1 (bn_stats) -> transpose via PE -> hT [1024,512] (per-partition affine)
  - qprojT/kprojT matmuls; rearranged into per-head [64, c, 128] blocks
    (qblk/kblk); vproj natural -> vaug with interleaved ones columns
  - scoresT per (head, e-block): 16 K=64 matmuls from qblk/kblk sub-blocks,
    + host-permuted mask add (DVE), exp (ACT, no max-subtraction: scores
    are bounded), then accumulate out^T = [V|1]^T @ expT into PSUM, which
    yields softmax numerator AND denominator in one accumulation.
  - transpose-back per 128-col block + normalize -> h_attn
  - residual + LN2 -> h2T; MLP: mlp1T = w1-chunks @ h2T, exact Gelu (ACT),
    mlp2 natural with ghT chunks as lhsT; final residual add, DMA out.
"""
import sys

sys.path.insert(0, "/opt/trn_rl_repo")

import numpy as np
import ml_dtypes

import concourse.bass as bass
import concourse.mybir as mybir
import concourse.tile as tile
from concourse.bass_utils import run_bass_kernel_spmd
from concourse.masks import make_identity

B, S, D = 2, 2048, 1024
H, HD, MLP = 16, 64, 4096
EPS = 1e-5
R = 512          # rows per core
NCORES = 8
F32 = mybir.dt.float32
BF16 = mybir.dt.bfloat16
BF = ml_dtypes.bfloat16
AF = mybir.ActivationFunctionType
AOP = mybir.AluOpType


# walrus in this container rejects >1 sync-wait on TPB_CTRL (Drain): split the
# TileContext final-drain waits across sequential drains (same AND semantics).
def _patch_drain():
    if getattr(tile.TileContext, "_dab_patched", False):
        return

    def _patched_dab(self, tick_clock, wait_clock):
        from concourse.vector_clock import ScopedClock
        drain_inst = self.nc.sync.drain()
        wait_clock.add_sem_waits(drain_inst.ins,
                                 ScopedClock({None: tick_clock.global_clock}))
        si = drain_inst.ins.sync_info
        if si is not None and len(si.on_wait) > 1:
            waits = list(si.on_wait)
            drain_inst.ins.sync_info = mybir.SyncInfo(on_wait=waits[:1],
                                                      on_update=list(si.on_update))
            for w in waits[1:]:
                extra = self.nc.sync.drain()
                extra.ins.sync_info = mybir.SyncInfo(on_wait=[w], on_update=[])
        self.nc.all_engine_barrier()
        assert self.sems is not None
        popped = self.nc._tile_sem_poison_stack.pop()
        assert popped is self._sem_poison
        self.nc.clear_and_free_semaphores(list(self.sems.allocated().values()))
        self.nc.all_engine_barrier()

    tile.TileContext._drain_and_barrier = _patched_dab
    tile.TileContext._dab_patched = True


# This walrus build accepts at most ONE sync-wait per instruction
# (setupSyncWait raises "Too many sync wait commands" otherwise).  Hoist
# excess waits onto same-engine NoOp carriers placed immediately before the
# instruction: engine streams execute in order, so waiting on the carrier
# then the instruction is equivalent to the instruction waiting on all.
_WAIT_LIMIT = 1


def _split_waits(nc):
    n_carriers = 0
    for bbname, bbw in nc.bb_map.items():
        il = bbw.bb.instructions
        out = []
        for inst in il:
            si = inst.sync_info
            if si is not None and len(si.on_wait) > _WAIT_LIMIT:
                waits = list(si.on_wait)
                extra, keep = waits[:-_WAIT_LIMIT], waits[-_WAIT_LIMIT:]
                for w in extra:
                    nop = mybir.InstNoOp(name=f"wsplit_{n_carriers}", ins=[], outs=[])
                    nop.engine = inst.engine
                    nop.sync_info = mybir.SyncInfo(on_wait=[w], on_update=[])
                    nc.register_instruction(nop, overwrite=True)
                    out.append(nop)
                    n_carriers += 1
                inst.sync_info = mybir.SyncInfo(on_wait=keep,
                                                on_update=list(si.on_update))
            out.append(inst)
        bbw.bb.instructions = out
    return n_carriers


def _phase_a(nc, tc, pp, pAB, consts, dram):
    """LN1 -> hT, QKV projections -> qblk/kblk/vaug. Returns nothing."""
    ident, eps_t, cols = consts["ident_bf"], consts["eps"], consts
    x_sb = consts["x_sb"]
    qblk, kblk, vaug, mask_sb = (consts["qblk"], consts["kblk"],
                                 consts["vaug"], consts["mask_sb"])
    with tc.tile_pool(name="phA", bufs=1) as pA, \
         tc.tile_pool(name="stat", bufs=8) as stat, \
         tc.tile_pool(name="psP", bufs=4, space="PSUM") as psP, \
         tc.tile_pool(name="wload", bufs=1) as wl, \
         tc.tile_pool(name="stg", bufs=2) as stg:
        xc = pA.tile([128, 4, D], BF16, tag="xc")
        hT = pA.tile([128, 8, R], BF16, tag="hT")
        x_r = dram["x"].rearrange("(n p) d -> n p d", p=128)
        for i in range(4):
            nc.sync.dma_start(x_sb[:, i, :], x_r[i])
            st = stat.tile([128, 2, 6], F32, tag="st")
            nc.vector.bn_stats(st[:, 0, :], x_sb[:, i, 0:512])
            nc.vector.bn_stats(st[:, 1, :], x_sb[:, i, 512:D])
            mv = stat.tile([128, 2], F32, tag="mv")
            nc.vector.bn_aggr(mv, st)
            sd = stat.tile([128, 1], F32, tag="sd")
            nc.scalar.activation(sd, mv[:, 1:2], AF.Sqrt, bias=eps_t, scale=1.0)
            rs = stat.tile([128, 1], F32, tag="rs")
            nc.vector.reciprocal(rs, sd)
            nc.vector.tensor_scalar(out=xc[:, i, :], in0=x_sb[:, i, :],
                                    scalar1=mv[:, 0:1], scalar2=rs,
                                    op0=AOP.subtract, op1=AOP.mult)
        for dt8 in range(8):
            for rt in range(4):
                nc.sync.dma_start(hT[:, dt8, 128 * rt:128 * rt + 128],
                                  xc[:, rt, 128 * dt8:128 * dt8 + 128],
                                  transpose=True)
            nc.vector.tensor_scalar(out=hT[:, dt8, :], in0=hT[:, dt8, :],
                                    scalar1=cols["g1"][:, dt8:dt8 + 1],
                                    scalar2=cols["b1"][:, dt8:dt8 + 1],
                                    op0=AOP.mult, op1=AOP.add)
        # Q and K projections (transposed layout) -> qblk/kblk
        for wname, bname, blk in (("wq", "bq", qblk), ("wk", "bk", kblk)):
            w_sb = wl.tile([128, 8, D], BF16, tag="w", name=f"w_{wname}")
            nc.sync.dma_start(w_sb, dram[wname].rearrange("(c p) d -> p c d", p=128))
            b_c = cols[bname]
            for t8 in range(8):
                ps = psP.tile([128, R], F32, tag="pp", name=f"ps_{wname}_{t8}")
                for ci in range(8):
                    nc.tensor.matmul(ps, w_sb[:, ci, 128 * t8:128 * t8 + 128],
                                     hT[:, ci, :], start=(ci == 0), stop=(ci == 7))
                sg = stg.tile([128, R], BF16, tag="sg", name=f"sg_{wname}_{t8}")
                nc.vector.tensor_scalar(out=sg, in0=ps, scalar1=b_c[:, t8:t8 + 1],
                                        scalar2=None, op0=AOP.add)
                nc.sync.dma_start(blk[:, :, 2 * t8, :],
                                  sg[0:64, :].rearrange("p (j r) -> p j r", j=4))
                nc.sync.dma_start(blk[:, :, 2 * t8 + 1, :],
                                  sg[64:128, :].rearrange("p (j r) -> p j r", j=4))
        # V projection (natural layout) -> vaug
        wv_sb = wl.tile([128, 8, D], BF16, tag="w")
        nc.sync.dma_start(wv_sb, dram["wv"].rearrange("(c p) d -> p c d", p=128))
        for rt in range(4):
            for hf in range(2):
                ps = psP.tile([128, R], F32, tag="pp", name=f"ps_v_{rt}_{hf}")
                for ci in range(8):
                    nc.tensor.matmul(ps, hT[:, ci, 128 * rt:128 * rt + 128],
                                     wv_sb[:, ci, 512 * hf:512 * hf + 512],
                                     start=(ci == 0), stop=(ci == 7))
                nc.vector.tensor_copy(vaug[:, rt, 8 * hf:8 * hf + 8, 0:64],
                                      ps.rearrange("p (e dd) -> p e dd", dd=64))


def _phase_b(nc, tc, consts, hattn):
    """Attention: scoresT -> mask+exp -> V_aug accumulation -> normalize."""
    ident = consts["ident"]
    qblk, kblk, vaug, mask_sb = (consts["qblk"], consts["kblk"],
                                 consts["vaug"], consts["mask_sb"])
    with tc.tile_pool(name="ex", bufs=2) as exp_pool, \
         tc.tile_pool(name="nmsb", bufs=2) as nms, \
         tc.tile_pool(name="scps", bufs=2, space="PSUM") as scps, \
         tc.tile_pool(name="outps", bufs=1, space="PSUM") as outps, \
         tc.tile_pool(name="attst", bufs=8) as attst:
        for j in range(4):
            outp = outps.tile([65, S], F32, tag="op", name=f"op_{j}")
            for e in range(16):
                ex = exp_pool.tile([128, S], BF16, tag="ex", name=f"ex_{j}_{e}")
                exr = exp_pool.tile([128, S], BF16, tag="exr", name=f"exr_{j}_{e}")
                for hf in range(2):
                    sc = scps.tile([128, 1024], F32, tag="sc",
                                   name=f"sc_{j}_{e}_{hf}")
                    for q2 in range(2):
                        col = 512 * q2
                        c0 = 8 * hf + 4 * q2
                        nc.tensor.matmul(sc[:, col:col + 512],
                                         kblk[:, j, e, :],
                                         qblk[:, j, c0:c0 + 4, :],
                                         start=True, stop=True)
                    nc.scalar.activation(exr[:, 1024 * hf:1024 * hf + 1024], sc, AF.Exp)
                    nc.vector.tensor_tensor(
                        out=ex[:, 1024 * hf:1024 * hf + 1024],
                        in0=exr[:, 1024 * hf:1024 * hf + 1024],
                        in1=mask_sb[:, e, 1024 * hf:1024 * hf + 1024],
                        op=AOP.mult)
                    for q2 in range(2):
                        col = 1024 * hf + 512 * q2
                        nc.tensor.matmul(outp[:, col:col + 512], vaug[:, j, e, :],
                                         ex[:, col:col + 512],
                                         start=(e == 0), stop=(e == 15))
            nm = nms.tile([65, S], F32, tag="nm", name=f"nm_{j}")
            nc.vector.tensor_copy(nm, outp)
            for c16 in range(16):
                tp = scps.tile([128, 1024], F32, tag="sc", name=f"tpb_{j}_{c16}")
                nc.tensor.transpose(tp[:, 0:65], nm[:, 128 * c16:128 * c16 + 128],
                                    ident[0:65, 0:65])
                rd = attst.tile([128, 1], F32, tag="rd", name=f"rd_{j}_{c16}")
                nc.vector.reciprocal(rd, tp[:, 64:65])
                nc.vector.tensor_scalar(out=hattn[:, j, 64 * c16:64 * c16 + 64],
                                        in0=tp[:, 0:64], scalar1=rd, scalar2=None,
                                        op0=AOP.mult)


def _phase_cd(nc, tc, consts, dram, hattn):
    """Residual + LN2 -> h2T; MLP; final residual; DMA out."""
    ident, eps_t = consts["ident"], consts["eps"]
    x_sb = consts["x_sb"]
    with tc.tile_pool(name="phC", bufs=1) as pC, \
         tc.tile_pool(name="stat2", bufs=8) as stat2, \
         tc.tile_pool(name="tmp2", bufs=2) as tmp2:
        xc2 = pC.tile([128, 4, D], F32, tag="xc2")
        h2T = pC.tile([128, 8, R], BF16, tag="h2T")
        for i in range(4):
            h2p = tmp2.tile([128, D], F32, tag="h2p", name=f"h2p_{i}")
            nc.vector.tensor_tensor(out=h2p, in0=hattn[:, i, :],
                                    in1=x_sb[:, i, :], op=AOP.add)
            nc.vector.tensor_tensor(out=h2p, in0=h2p, in1=consts["bv_b"], op=AOP.add)
            st = stat2.tile([128, 2, 6], F32, tag="st", name=f"st2_{i}")
            nc.vector.bn_stats(st[:, 0, :], h2p[:, 0:512])
            nc.vector.bn_stats(st[:, 1, :], h2p[:, 512:D])
            mv = stat2.tile([128, 2], F32, tag="mv", name=f"mv2_{i}")
            nc.vector.bn_aggr(mv, st)
            sd = stat2.tile([128, 1], F32, tag="sd", name=f"sd2_{i}")
            nc.scalar.activation(sd, mv[:, 1:2], AF.Sqrt, bias=eps_t, scale=1.0)
            rs = stat2.tile([128, 1], F32, tag="rs", name=f"rs2_{i}")
            nc.vector.reciprocal(rs, sd)
            nc.vector.tensor_scalar(out=xc2[:, i, :], in0=h2p,
                                    scalar1=mv[:, 0:1], scalar2=rs,
                                    op0=AOP.subtract, op1=AOP.mult)
        xc2b = pC.tile([128, 4, D], BF16, tag="xc2b")
        for rt in range(4):
            nc.vector.tensor_copy(xc2b[:, rt, :], xc2[:, rt, :])
        for dt8 in range(8):
            for rt in range(4):
                nc.sync.dma_start(h2T[:, dt8, 128 * rt:128 * rt + 128],
                                  xc2b[:, rt, 128 * dt8:128 * dt8 + 128],
                                  transpose=True)
            nc.vector.tensor_scalar(out=h2T[:, dt8, :], in0=h2T[:, dt8, :],
                                    scalar1=consts["g2"][:, dt8:dt8 + 1],
                                    scalar2=consts["b2l"][:, dt8:dt8 + 1],
                                    op0=AOP.mult, op1=AOP.add)
        ghT = pC.tile([128, 32, R], BF16, tag="ghT")
        with tc.tile_pool(name="w1l", bufs=3) as w1l, \
             tc.tile_pool(name="psD1", bufs=4, space="PSUM") as psD1:
            for mt in range(32):
                w1t = w1l.tile([128, 8, 128], BF16, tag="w1", name=f"w1t_{mt}")
                nc.sync.dma_start(w1t, dram["w1"][mt])
                ps = psD1.tile([128, R], F32, tag="m1", name=f"m1_{mt}")
                for ci in range(8):
                    nc.tensor.matmul(ps, w1t[:, ci, :], h2T[:, ci, :],
                                     start=(ci == 0), stop=(ci == 7))
                nc.scalar.activation(ghT[:, mt, :], ps, AF.Gelu,
                                     bias=consts["b1m"][:, mt:mt + 1], scale=1.0)
        resf = pC.tile([128, 4, D], F32, tag="resf")
        for rt in range(4):
            nc.vector.tensor_tensor(out=resf[:, rt, :], in0=xc2[:, rt, :],
                                    in1=consts["g2_b"], op=AOP.mult)
            nc.vector.tensor_tensor(out=resf[:, rt, :], in0=resf[:, rt, :],
                                    in1=consts["bcomb_b"], op=AOP.add)
        ostg = pC.tile([128, 4, D], F32, tag="ostg")
        out_r = dram["out"].rearrange("(n p) d -> n p d", p=128)
        with tc.tile_pool(name="w2l", bufs=3) as w2l, \
             tc.tile_pool(name="psD2", bufs=1, space="PSUM") as psD2:
            m2ps = [[psD2.tile([128, 512], F32, tag=f"m2_{rt}_{dc}",
                               name=f"m2_{rt}_{dc}")
                     for dc in range(2)] for rt in range(4)]
            for mc in range(32):
                w2t = w2l.tile([128, D], BF16, tag="w2", name=f"w2t_{mc}")
                nc.sync.dma_start(w2t, dram["w2"][128 * mc:128 * mc + 128, :])
                for rt in range(4):
                    for dc in range(2):
                        nc.tensor.matmul(m2ps[rt][dc],
                                         ghT[:, mc, 128 * rt:128 * rt + 128],
                                         w2t[:, 512 * dc:512 * dc + 512],
                                         start=(mc == 0), stop=(mc == 31))
            for rt in range(4):
                for dc in range(2):
                    nc.vector.tensor_tensor(out=ostg[:, rt, 512 * dc:512 * dc + 512],
                                            in0=m2ps[rt][dc],
                                            in1=resf[:, rt, 512 * dc:512 * dc + 512],
                                            op=AOP.add)
                nc.sync.dma_start(out_r[rt], ostg[:, rt, :])


def _build_program():
    _patch_drain()
    nc = bass.Bass()
    dram = {
        "x": nc.dram_tensor("x", [R, D], F32, kind="ExternalInput"),
        "maskp": nc.dram_tensor("maskp", [S, S], BF16, kind="ExternalInput"),
        "wq": nc.dram_tensor("wq", [D, D], BF16, kind="ExternalInput"),
        "wk": nc.dram_tensor("wk", [D, D], BF16, kind="ExternalInput"),
        "wv": nc.dram_tensor("wv", [D, D], BF16, kind="ExternalInput"),
        "bq": nc.dram_tensor("bq", [D], F32, kind="ExternalInput"),
        "bk": nc.dram_tensor("bk", [D], F32, kind="ExternalInput"),
        "bv": nc.dram_tensor("bv", [D], F32, kind="ExternalInput"),
        "g1": nc.dram_tensor("g1", [D], F32, kind="ExternalInput"),
        "b1": nc.dram_tensor("b1", [D], F32, kind="ExternalInput"),
        "g2": nc.dram_tensor("g2", [D], F32, kind="ExternalInput"),
        "b2l": nc.dram_tensor("b2l", [D], F32, kind="ExternalInput"),
        "bcomb": nc.dram_tensor("bcomb", [D], F32, kind="ExternalInput"),
        "w1": nc.dram_tensor("w1", [32, 128, 8, 128], BF16, kind="ExternalInput"),
        "b1m": nc.dram_tensor("b1m", [MLP], F32, kind="ExternalInput"),
        "w2": nc.dram_tensor("w2", [MLP, D], BF16, kind="ExternalInput"),
        "out": nc.dram_tensor("out", [R, D], F32, kind="ExternalOutput"),
    }
    with tile.TileContext(nc) as tc:
        with tc.tile_pool(name="persist", bufs=1) as pp:
            consts = {}
            ident = pp.tile([128, 128], F32, tag="ident")
            make_identity(nc, ident)
            consts["ident"] = ident
            ident_bf = pp.tile([128, 128], BF16, tag="identbf")
            make_identity(nc, ident_bf)
            consts["ident_bf"] = ident_bf
            eps_t = pp.tile([128, 1], F32, tag="eps")
            nc.vector.memset(eps_t, EPS)
            consts["eps"] = eps_t

            def pcol(key, n):
                t_ = pp.tile([128, n], F32, tag=f"c_{key}", name=f"c_{key}")
                nc.sync.dma_start(t_, dram[key].rearrange("(t p) -> p t", p=128))
                return t_

            for key, n in (("bq", 8), ("bk", 8), ("g1", 8), ("b1", 8),
                           ("g2", 8), ("b2l", 8), ("b1m", 32)):
                consts[key] = pcol(key, n)

            def bcast(key):
                t_ = pp.tile([128, D], F32, tag=f"b_{key}", name=f"b_{key}")
                src = dram[key][:]
                bc = bass.AP(tensor=src.tensor, offset=src.offset,
                             ap=[[0, 128]] + [list(a) for a in src.ap])
                nc.sync.dma_start(t_, bc)
                return t_

            consts["bv_b"] = bcast("bv")
            consts["g2_b"] = bcast("g2")
            consts["bcomb_b"] = bcast("bcomb")

            consts["x_sb"] = pp.tile([128, 4, D], F32, tag="x", name="x_sb")

            with tc.tile_pool(name="poolBC", bufs=1) as pBC:
                with tc.tile_pool(name="poolAB", bufs=1) as pAB:
                    consts["qblk"] = pAB.tile([64, 4, 16, 128], BF16, tag="qblk",
                                              name="qblk")   # [d, j, c, rr]
                    consts["kblk"] = pAB.tile([64, 4, 16, 128], BF16, tag="kblk",
                                              name="kblk")
                    consts["vaug"] = pAB.tile([128, 4, 16, 65], BF16, tag="vaug",
                                              name="vaug")   # [rr, j, e, d|1]
                    nc.vector.memset(consts["vaug"][:, :, :, 64:65], 1.0)
                    consts["mask_sb"] = pAB.tile([128, 16, S], BF16, tag="mask",
                                                 name="mask_sb")
                    _phase_a(nc, tc, pp, pAB, consts, dram)
                    mask_r = dram["maskp"].rearrange("(e p) s -> e p s", p=128)
                    for e in range(16):
                        nc.sync.dma_start(consts["mask_sb"][:, e, :], mask_r[e])
                    hattn = pBC.tile([128, 4, D], F32, tag="hattn", name="hattn")
                    _phase_b(nc, tc, consts, hattn)
                _phase_cd(nc, tc, consts, dram, hattn)
    n = _split_waits(nc)
    print(f"[kernel] split {n} excess sync-waits onto NoOp carriers")
    return nc


_PROGRAM = None


def _get_program():
    global _PROGRAM
    if _PROGRAM is None:
        _PROGRAM = _build_program()
    return _PROGRAM


def _prep_inputs(x, dis_attn_mask, cls_attn_mask, wq, bq, wk, bk, wv, bv,
                 ln1_g, ln1_b, ln2_g, ln2_b, w1, b1, w2, b2):
    """Host-side prep: per-core shards + weight dtype/layout conversion."""
    x = np.asarray(x, np.float32)
    wq8 = (np.asarray(wq, np.float32) / 8.0).astype(BF)
    bq8 = (np.asarray(bq, np.float32) / 8.0)
    wk_b = np.asarray(wk, np.float32).astype(BF)
    wv_b = np.asarray(wv, np.float32).astype(BF)
    # w1 pre-tiled to [32(mt), 128(p=din%128), 8(c=din//128), 128(n=mout%128)]
    w1_t = np.ascontiguousarray(
        np.asarray(w1, np.float32).astype(BF).reshape(8, 128, 32, 128).transpose(2, 1, 0, 3))
    w2_b = np.asarray(w2, np.float32).astype(BF)
    bcomb = (np.asarray(ln2_b, np.float32) + np.asarray(b2, np.float32))
    masks = {}
    for bb in range(B):
        msum = (np.asarray(dis_attn_mask[bb], np.float32)
                + np.asarray(cls_attn_mask[bb], np.float32))
        masks[bb] = np.exp(np.ascontiguousarray(
            msum.reshape(128, 16, 128, 16).transpose(3, 2, 1, 0).reshape(S, S))).astype(BF)
    shared = {
        "wq": wq8, "wk": wk_b, "wv": wv_b,
        "bq": bq8.astype(np.float32), "bk": np.asarray(bk, np.float32),
        "bv": np.asarray(bv, np.float32),
        "g1": np.asarray(ln1_g, np.float32), "b1": np.asarray(ln1_b, np.float32),
        "g2": np.asarray(ln2_g, np.float32), "b2l": np.asarray(ln2_b, np.float32),
        "bcomb": bcomb.astype(np.float32),
        "w1": w1_t, "b1m": np.asarray(b1, np.float32), "w2": w2_b,
    }
    in_maps = []
    for core in range(NCORES):
        bb = core // 4
        g = core % 4
        m = dict(shared)
        m["x"] = np.ascontiguousarray(x[bb, 512 * g:512 * g + 512])
        m["maskp"] = masks[bb]
        in_maps.append(m)
    return in_maps


def kernel(**inputs):
    nc = _get_program()
    in_maps = _prep_inputs(**inputs)
    res = run_bass_kernel_spmd(nc, in_maps, core_ids=list(range(NCORES)))
    out = np.zeros((B, S, D), np.float32)
    for core in range(NCORES):
        bb = core // 4
        g = core % 4
        out[bb, 512 * g:512 * g + 512] = res.results[core]["out"]
    return out


if __name__ == "__main__":
    sys.path.insert(0, "/root/problem")
    import reference
    inputs = {k: np.asarray(v) for k, v in reference.setup_inputs().items()}
    expected = np.asarray(reference.reference(**inputs))
    actual = kernel(**inputs)
    err = np.abs(actual - expected)
    scale = np.abs(expected).max()
    print("max abs err:", err.max(), "scale:", scale, "rel:", err.max() / scale)


# revision 12
# speedup vs baseline: 1.0843x; 1.0010x over previous
"""Trainium2 Bass kernel for nn_AttentionBlock (B=2,S=2048,D=1024,H=16,MLP=4096).

Key structural insight: the reference does q.reshape(B,H,S,HD) on a row-major
[B,S,D] tensor (no transpose), so head hh consumes ROWS 128*hh:128*(hh+1) of
the projected [2048,1024] matrix reinterpreted as [2048,64]. The entire block
(LN1 -> QKV -> attention -> residual -> LN2 -> MLP -> residual) therefore
decomposes into 32 independent 128-row blocks (B*H), aligned across all ops.
Sharding: 8 cores x 4 head-blocks (512 rows) each, zero collectives.

Per-core device program (all matmuls bf16, accum f32):
  - LN1 (bn_stats) -> transpose via PE -> hT [1024,512] (per-partition affine)
  - qprojT/kprojT matmuls; rearranged into per-head [64, c, 128] blocks
    (qblk/kblk); vproj natural -> vaug with interleaved ones columns
  - scoresT per (head, e-block): 16 K=64 matmuls from qblk/kblk sub-blocks,
    + host-permuted mask add (DVE), exp (ACT, no max-subtraction: scores
    are bounded), then accumulate out^T = [V|1]^T @ expT into PSUM, which
    yields softmax numerator AND denominator in one accumulation.
  - transpose-back per 128-col block + normalize -> h_attn
  - residual + LN2 -> h2T; MLP: mlp1T = w1-chunks @ h2T, exact Gelu (ACT),
    mlp2 natural with ghT chunks as lhsT; final residual add, DMA out.
"""
import sys

sys.path.insert(0, "/opt/trn_rl_repo")

import numpy as np
import ml_dtypes

import concourse.bass as bass
import concourse.mybir as mybir
import concourse.tile as tile
from concourse.bass_utils import run_bass_kernel_spmd
from concourse.masks import make_identity

B, S, D = 2, 2048, 1024
H, HD, MLP = 16, 64, 4096
EPS = 1e-5
R = 512          # rows per core
NCORES = 8
F32 = mybir.dt.float32
BF16 = mybir.dt.bfloat16
BF = ml_dtypes.bfloat16
AF = mybir.ActivationFunctionType
AOP = mybir.AluOpType


# walrus in this container rejects >1 sync-wait on TPB_CTRL (Drain): split the
# TileContext final-drain waits across sequential drains (same AND semantics).
def _patch_drain():
    if getattr(tile.TileContext, "_dab_patched", False):
        return

    def _patched_dab(self, tick_clock, wait_clock):
        from concourse.vector_clock import ScopedClock
        drain_inst = self.nc.sync.drain()
        wait_clock.add_sem_waits(drain_inst.ins,
                                 ScopedClock({None: tick_clock.global_clock}))
        si = drain_inst.ins.sync_info
        if si is not None and len(si.on_wait) > 1:
            waits = list(si.on_wait)
            drain_inst.ins.sync_info = mybir.SyncInfo(on_wait=waits[:1],
                                                      on_update=list(si.on_update))
            for w in waits[1:]:
                extra = self.nc.sync.drain()
                extra.ins.sync_info = mybir.SyncInfo(on_wait=[w], on_update=[])
        self.nc.all_engine_barrier()
        assert self.sems is not None
        popped = self.nc._tile_sem_poison_stack.pop()
        assert popped is self._sem_poison
        self.nc.clear_and_free_semaphores(list(self.sems.allocated().values()))
        self.nc.all_engine_barrier()

    tile.TileContext._drain_and_barrier = _patched_dab
    tile.TileContext._dab_patched = True


# This walrus build accepts at most ONE sync-wait per instruction
# (setupSyncWait raises "Too many sync wait commands" otherwise).  Hoist
# excess waits onto same-engine NoOp carriers placed immediately before the
# instruction: engine streams execute in order, so waiting on the carrier
# then the instruction is equivalent to the instruction waiting on all.
_WAIT_LIMIT = 1


def _split_waits(nc):
    n_carriers = 0
    for bbname, bbw in nc.bb_map.items():
        il = bbw.bb.instructions
        out = []
        for inst in il:
            si = inst.sync_info
            if si is not None and len(si.on_wait) > _WAIT_LIMIT:
                waits = list(si.on_wait)
                extra, keep = waits[:-_WAIT_LIMIT], waits[-_WAIT_LIMIT:]
                for w in extra:
                    nop = mybir.InstNoOp(name=f"wsplit_{n_carriers}", ins=[], outs=[])
                    nop.engine = inst.engine
                    nop.sync_info = mybir.SyncInfo(on_wait=[w], on_update=[])
                    nc.register_instruction(nop, overwrite=True)
                    out.append(nop)
                    n_carriers += 1
                inst.sync_info = mybir.SyncInfo(on_wait=keep,
                                                on_update=list(si.on_update))
            out.append(inst)
        bbw.bb.instructions = out
    return n_carriers


def _phase_a(nc, tc, pp, pAB, consts, dram):
    """LN1 -> hT, QKV projections -> qblk/kblk/vaug. Returns nothing."""
    ident, eps_t, cols = consts["ident_bf"], consts["eps"], consts
    x_sb = consts["x_sb"]
    qblk, kblk, vaug, mask_sb = (consts["qblk"], consts["kblk"],
                                 consts["vaug"], consts["mask_sb"])
    with tc.tile_pool(name="phA", bufs=1) as pA, \
         tc.tile_pool(name="stat", bufs=8) as stat, \
         tc.tile_pool(name="psP", bufs=4, space="PSUM") as psP, \
         tc.tile_pool(name="wload", bufs=1) as wl, \
         tc.tile_pool(name="stg", bufs=2) as stg:
        xc = pA.tile([128, 4, D], BF16, tag="xc")
        hT = pA.tile([128, 8, R], BF16, tag="hT")
        nc.sync.dma_start(x_sb, dram["x"].rearrange("(n p) d -> p n d", p=128))
        for i in range(4):
            st = stat.tile([128, 2, 6], F32, tag="st")
            nc.vector.bn_stats(st[:, 0, :], x_sb[:, i, 0:512])
            nc.vector.bn_stats(st[:, 1, :], x_sb[:, i, 512:D])
            mv = stat.tile([128, 2], F32, tag="mv")
            nc.vector.bn_aggr(mv, st)
            sd = stat.tile([128, 1], F32, tag="sd")
            nc.scalar.activation(sd, mv[:, 1:2], AF.Sqrt, bias=eps_t, scale=1.0)
            rs = stat.tile([128, 1], F32, tag="rs")
            nc.vector.reciprocal(rs, sd)
            nc.vector.tensor_scalar(out=xc[:, i, :], in0=x_sb[:, i, :],
                                    scalar1=mv[:, 0:1], scalar2=rs,
                                    op0=AOP.subtract, op1=AOP.mult)
        for rt in range(4):
            nc.sync.dma_start(hT[:, :, 128 * rt:128 * rt + 128],
                              xc[:, rt, :], transpose=True)
        for dt8 in range(8):
            nc.vector.tensor_scalar(out=hT[:, dt8, :], in0=hT[:, dt8, :],
                                    scalar1=cols["g1"][:, dt8:dt8 + 1],
                                    scalar2=cols["b1"][:, dt8:dt8 + 1],
                                    op0=AOP.mult, op1=AOP.add)
        # Q and K projections (transposed layout) -> qblk/kblk
        for wname, bname, blk in (("wq", "bq", qblk), ("wk", "bk", kblk)):
            w_sb = wl.tile([128, 8, D], BF16, tag="w", name=f"w_{wname}")
            nc.sync.dma_start(w_sb, dram[wname].rearrange("(c p) d -> p c d", p=128))
            b_c = cols[bname]
            for t8 in range(8):
                ps = psP.tile([128, R], F32, tag="pp", name=f"ps_{wname}_{t8}")
                for ci in range(8):
                    nc.tensor.matmul(ps, w_sb[:, ci, 128 * t8:128 * t8 + 128],
                                     hT[:, ci, :], start=(ci == 0), stop=(ci == 7))
                sg = stg.tile([128, R], BF16, tag="sg", name=f"sg_{wname}_{t8}")
                nc.vector.tensor_scalar(out=sg, in0=ps, scalar1=b_c[:, t8:t8 + 1],
                                        scalar2=None, op0=AOP.add)
                nc.gpsimd.dma_start(blk[:, :, 2 * t8, :],
                                     sg[0:64, :].rearrange("p (j r) -> p j r", j=4))
                nc.gpsimd.dma_start(blk[:, :, 2 * t8 + 1, :],
                                    sg[64:128, :].rearrange("p (j r) -> p j r", j=4))
        # V projection (natural layout) -> vaug
        wv_sb = wl.tile([128, 8, D], BF16, tag="w")
        nc.sync.dma_start(wv_sb, dram["wv"].rearrange("(c p) d -> p c d", p=128))
        for rt in range(4):
            for hf in range(2):
                ps = psP.tile([128, R], F32, tag="pp", name=f"ps_v_{rt}_{hf}")
                for ci in range(8):
                    nc.tensor.matmul(ps, hT[:, ci, 128 * rt:128 * rt + 128],
                                     wv_sb[:, ci, 512 * hf:512 * hf + 512],
                                     start=(ci == 0), stop=(ci == 7))
                nc.vector.tensor_copy(vaug[:, rt, 8 * hf:8 * hf + 8, 0:64],
                                      ps.rearrange("p (e dd) -> p e dd", dd=64))


def _phase_b(nc, tc, consts, hattn):
    """Attention: scoresT -> mask+exp -> V_aug accumulation -> normalize."""
    ident = consts["ident"]
    qblk, kblk, vaug, mask_sb = (consts["qblk"], consts["kblk"],
                                 consts["vaug"], consts["mask_sb"])
    with tc.tile_pool(name="ex", bufs=2) as exp_pool, \
         tc.tile_pool(name="nmsb", bufs=2) as nms, \
         tc.tile_pool(name="scps", bufs=4, space="PSUM") as scps, \
         tc.tile_pool(name="outps", bufs=1, space="PSUM") as outps, \
         tc.tile_pool(name="attst", bufs=8) as attst:
        for j in range(4):
            outp = outps.tile([65, S], F32, tag="op", name=f"op_{j}")
            for e in range(16):
                ex = exp_pool.tile([128, S], BF16, tag="ex", name=f"ex_{j}_{e}")
                for q in range(4):
                    col = 512 * q
                    sc = scps.tile([128, 512], F32, tag="sc",
                                   name=f"sc_{j}_{e}_{q}")
                    nc.tensor.matmul(sc, kblk[:, j, e, :],
                                     qblk[:, j, 4 * q:4 * q + 4, :],
                                     start=True, stop=True)
                    nc.scalar.activation(ex[:, col:col + 512], sc, AF.Exp)
                    nc.vector.tensor_tensor(
                        out=ex[:, col:col + 512],
                        in0=ex[:, col:col + 512],
                        in1=mask_sb[:, e, col:col + 512],
                        op=AOP.mult)
                    nc.tensor.matmul(outp[:, col:col + 512], vaug[:, j, e, :],
                                     ex[:, col:col + 512],
                                     start=(e == 0), stop=(e == 15))
            nm = nms.tile([65, S], F32, tag="nm", name=f"nm_{j}")
            nc.vector.tensor_copy(nm, outp)
            for c16 in range(16):
                tp = scps.tile([128, 512], F32, tag="sc", name=f"tpb_{j}_{c16}")
                nc.tensor.transpose(tp[:, 0:65], nm[:, 128 * c16:128 * c16 + 128],
                                    ident[0:65, 0:65])
                rd = attst.tile([128, 1], F32, tag="rd", name=f"rd_{j}_{c16}")
                nc.vector.reciprocal(rd, tp[:, 64:65])
                nc.vector.tensor_scalar(out=hattn[:, j, 64 * c16:64 * c16 + 64],
                                        in0=tp[:, 0:64], scalar1=rd, scalar2=None,
                                        op0=AOP.mult)


def _phase_cd(nc, tc, consts, dram, hattn):
    """Residual + LN2 -> h2T; MLP; final residual; DMA out."""
    ident, eps_t = consts["ident"], consts["eps"]
    x_sb = consts["x_sb"]
    with tc.tile_pool(name="phC", bufs=1) as pC, \
         tc.tile_pool(name="stat2", bufs=8) as stat2, \
         tc.tile_pool(name="tmp2", bufs=2) as tmp2:
        xc2 = pC.tile([128, 4, D], F32, tag="xc2")
        h2T = pC.tile([128, 8, R], BF16, tag="h2T")
        for i in range(4):
            h2p = tmp2.tile([128, D], F32, tag="h2p", name=f"h2p_{i}")
            nc.vector.tensor_tensor(out=h2p, in0=hattn[:, i, :],
                                    in1=x_sb[:, i, :], op=AOP.add)
            nc.vector.tensor_tensor(out=h2p, in0=h2p, in1=consts["bv_b"], op=AOP.add)
            st = stat2.tile([128, 2, 6], F32, tag="st", name=f"st2_{i}")
            nc.vector.bn_stats(st[:, 0, :], h2p[:, 0:512])
            nc.vector.bn_stats(st[:, 1, :], h2p[:, 512:D])
            mv = stat2.tile([128, 2], F32, tag="mv", name=f"mv2_{i}")
            nc.vector.bn_aggr(mv, st)
            sd = stat2.tile([128, 1], F32, tag="sd", name=f"sd2_{i}")
            nc.scalar.activation(sd, mv[:, 1:2], AF.Sqrt, bias=eps_t, scale=1.0)
            rs = stat2.tile([128, 1], F32, tag="rs", name=f"rs2_{i}")
            nc.vector.reciprocal(rs, sd)
            nc.vector.tensor_scalar(out=xc2[:, i, :], in0=h2p,
                                    scalar1=mv[:, 0:1], scalar2=rs,
                                    op0=AOP.subtract, op1=AOP.mult)
        xc2b = pC.tile([128, 4, D], BF16, tag="xc2b")
        for rt in range(4):
            nc.vector.tensor_copy(xc2b[:, rt, :], xc2[:, rt, :])
        for rt in range(4):
            nc.sync.dma_start(h2T[:, :, 128 * rt:128 * rt + 128],
                              xc2b[:, rt, :], transpose=True)
        for dt8 in range(8):
            nc.vector.tensor_scalar(out=h2T[:, dt8, :], in0=h2T[:, dt8, :],
                                    scalar1=consts["g2"][:, dt8:dt8 + 1],
                                    scalar2=consts["b2l"][:, dt8:dt8 + 1],
                                    op0=AOP.mult, op1=AOP.add)
        ghT = pC.tile([128, 32, R], BF16, tag="ghT")
        with tc.tile_pool(name="w1l", bufs=2) as w1l, \
             tc.tile_pool(name="psD1", bufs=4, space="PSUM") as psD1:
            for mt4 in range(8):
                w1t = w1l.tile([128, 4, 8, 128], BF16, tag="w1", name=f"w1t_{mt4}")
                nc.scalar.dma_start(
                    w1t, dram["w1"][4 * mt4:4 * mt4 + 4].rearrange("m p c n -> p m c n"))
                for sub in range(4):
                    mt = 4 * mt4 + sub
                    ps = psD1.tile([128, R], F32, tag="m1", name=f"m1_{mt}")
                    for ci in range(8):
                        nc.tensor.matmul(ps, w1t[:, sub, ci, :], h2T[:, ci, :],
                                         start=(ci == 0), stop=(ci == 7))
                    nc.scalar.activation(ghT[:, mt, :], ps, AF.Gelu,
                                         bias=consts["b1m"][:, mt:mt + 1], scale=1.0)
        resf = pC.tile([128, 4, D], F32, tag="resf")
        for rt in range(4):
            nc.vector.tensor_tensor(out=resf[:, rt, :], in0=xc2[:, rt, :],
                                    in1=consts["g2_b"], op=AOP.mult)
            nc.vector.tensor_tensor(out=resf[:, rt, :], in0=resf[:, rt, :],
                                    in1=consts["bcomb_b"], op=AOP.add)
        ostg = pC.tile([128, 4, D], F32, tag="ostg")
        out_r = dram["out"].rearrange("(n p) d -> n p d", p=128)
        with tc.tile_pool(name="w2l", bufs=2) as w2l, \
             tc.tile_pool(name="psD2", bufs=1, space="PSUM") as psD2:
            m2ps = [[psD2.tile([128, 512], F32, tag=f"m2_{rt}_{dc}",
                               name=f"m2_{rt}_{dc}")
                     for dc in range(2)] for rt in range(4)]
            for mc4 in range(8):
                w2t = w2l.tile([128, 4, D], BF16, tag="w2", name=f"w2t_{mc4}")
                nc.scalar.dma_start(
                    w2t, dram["w2"][512 * mc4:512 * mc4 + 512, :].rearrange(
                        "(m p) d -> p m d", p=128))
                for sub in range(4):
                    mc = 4 * mc4 + sub
                    for rt in range(4):
                        for dc in range(2):
                            nc.tensor.matmul(m2ps[rt][dc],
                                             ghT[:, mc, 128 * rt:128 * rt + 128],
                                             w2t[:, sub, 512 * dc:512 * dc + 512],
                                             start=(mc == 0), stop=(mc == 31))
            for rt in range(4):
                for dc in range(2):
                    nc.vector.tensor_tensor(out=ostg[:, rt, 512 * dc:512 * dc + 512],
                                            in0=m2ps[rt][dc],
                                            in1=resf[:, rt, 512 * dc:512 * dc + 512],
                                            op=AOP.add)
                nc.sync.dma_start(out_r[rt], ostg[:, rt, :])


def _build_program():
    _patch_drain()
    nc = bass.Bass()
    dram = {
        "x": nc.dram_tensor("x", [R, D], F32, kind="ExternalInput"),
        "maskp": nc.dram_tensor("maskp", [S, S], BF16, kind="ExternalInput"),
        "wq": nc.dram_tensor("wq", [D, D], BF16, kind="ExternalInput"),
        "wk": nc.dram_tensor("wk", [D, D], BF16, kind="ExternalInput"),
        "wv": nc.dram_tensor("wv", [D, D], BF16, kind="ExternalInput"),
        "bq": nc.dram_tensor("bq", [D], F32, kind="ExternalInput"),
        "bk": nc.dram_tensor("bk", [D], F32, kind="ExternalInput"),
        "bv": nc.dram_tensor("bv", [D], F32, kind="ExternalInput"),
        "g1": nc.dram_tensor("g1", [D], F32, kind="ExternalInput"),
        "b1": nc.dram_tensor("b1", [D], F32, kind="ExternalInput"),
        "g2": nc.dram_tensor("g2", [D], F32, kind="ExternalInput"),
        "b2l": nc.dram_tensor("b2l", [D], F32, kind="ExternalInput"),
        "bcomb": nc.dram_tensor("bcomb", [D], F32, kind="ExternalInput"),
        "w1": nc.dram_tensor("w1", [32, 128, 8, 128], BF16, kind="ExternalInput"),
        "b1m": nc.dram_tensor("b1m", [MLP], F32, kind="ExternalInput"),
        "w2": nc.dram_tensor("w2", [MLP, D], BF16, kind="ExternalInput"),
        "out": nc.dram_tensor("out", [R, D], F32, kind="ExternalOutput"),
    }
    with tile.TileContext(nc) as tc:
        with tc.tile_pool(name="persist", bufs=1) as pp:
            consts = {}
            ident = pp.tile([128, 128], F32, tag="ident")
            make_identity(nc, ident)
            consts["ident"] = ident
            ident_bf = pp.tile([128, 128], BF16, tag="identbf")
            make_identity(nc, ident_bf)
            consts["ident_bf"] = ident_bf
            eps_t = pp.tile([128, 1], F32, tag="eps")
            nc.vector.memset(eps_t, EPS)
            consts["eps"] = eps_t

            def pcol(key, n):
                t_ = pp.tile([128, n], F32, tag=f"c_{key}", name=f"c_{key}")
                nc.sync.dma_start(t_, dram[key].rearrange("(t p) -> p t", p=128))
                return t_

            for key, n in (("bq", 8), ("bk", 8), ("g1", 8), ("b1", 8),
                           ("g2", 8), ("b2l", 8), ("b1m", 32)):
                consts[key] = pcol(key, n)

            def bcast(key):
                t_ = pp.tile([128, D], F32, tag=f"b_{key}", name=f"b_{key}")
                src = dram[key][:]
                bc = bass.AP(tensor=src.tensor, offset=src.offset,
                             ap=[[0, 128]] + [list(a) for a in src.ap])
                nc.sync.dma_start(t_, bc)
                return t_

            consts["bv_b"] = bcast("bv")
            consts["g2_b"] = bcast("g2")
            consts["bcomb_b"] = bcast("bcomb")

            consts["x_sb"] = pp.tile([128, 4, D], F32, tag="x", name="x_sb")

            with tc.tile_pool(name="poolBC", bufs=1) as pBC:
                with tc.tile_pool(name="poolAB", bufs=1) as pAB:
                    consts["qblk"] = pAB.tile([64, 4, 16, 128], BF16, tag="qblk",
                                              name="qblk")   # [d, j, c, rr]
                    consts["kblk"] = pAB.tile([64, 4, 16, 128], BF16, tag="kblk",
                                              name="kblk")
                    consts["vaug"] = pAB.tile([128, 4, 16, 65], BF16, tag="vaug",
                                              name="vaug")   # [rr, j, e, d|1]
                    nc.vector.memset(consts["vaug"][:, :, :, 64:65], 1.0)
                    consts["mask_sb"] = pAB.tile([128, 16, S], BF16, tag="mask",
                                                 name="mask_sb")
                    _phase_a(nc, tc, pp, pAB, consts, dram)
                    mask_r = dram["maskp"].rearrange("(g e p) s -> g p e s", p=128, e=4)
                    for g in range(4):
                        nc.scalar.dma_start(consts["mask_sb"][:, 4 * g:4 * g + 4, :],
                                            mask_r[g])
                    hattn = pBC.tile([128, 4, D], F32, tag="hattn", name="hattn")
                    _phase_b(nc, tc, consts, hattn)
                _phase_cd(nc, tc, consts, dram, hattn)
    n = _split_waits(nc)
    print(f"[kernel] split {n} excess sync-waits onto NoOp carriers")
    return nc


_PROGRAM = None


def _get_program():
    global _PROGRAM
    if _PROGRAM is None:
        _PROGRAM = _build_program()
    return _PROGRAM


def _prep_inputs(x, dis_attn_mask, cls_attn_mask, wq, bq, wk, bk, wv, bv,
                 ln1_g, ln1_b, ln2_g, ln2_b, w1, b1, w2, b2):
    """Host-side prep: per-core shards + weight dtype/layout conversion."""
    x = np.asarray(x, np.float32)
    wq8 = (np.asarray(wq, np.float32) / 8.0).astype(BF)
    bq8 = (np.asarray(bq, np.float32) / 8.0)
    wk_b = np.asarray(wk, np.float32).astype(BF)
    wv_b = np.asarray(wv, np.float32).astype(BF)
    # w1 pre-tiled to [32(mt), 128(p=din%128), 8(c=din//128), 128(n=mout%128)]
    w1_t = np.ascontiguousarray(
        np.asarray(w1, np.float32).astype(BF).reshape(8, 128, 32, 128).transpose(2, 1, 0, 3))
    w2_b = np.asarray(w2, np.float32).astype(BF)
    bcomb = (np.asarray(ln2_b, np.float32) + np.asarray(b2, np.float32))
    masks = {}
    for bb in range(B):
        msum = (np.asarray(dis_attn_mask[bb], np.float32)
                + np.asarray(cls_attn_mask[bb], np.float32))
        masks[bb] = np.exp(np.ascontiguousarray(
            msum.reshape(128, 16, 128, 16).transpose(3, 2, 1, 0).reshape(S, S))).astype(BF)
    shared = {
        "wq": wq8, "wk": wk_b, "wv": wv_b,
        "bq": bq8.astype(np.float32), "bk": np.asarray(bk, np.float32),
        "bv": np.asarray(bv, np.float32),
        "g1": np.asarray(ln1_g, np.float32), "b1": np.asarray(ln1_b, np.float32),
        "g2": np.asarray(ln2_g, np.float32), "b2l": np.asarray(ln2_b, np.float32),
        "bcomb": bcomb.astype(np.float32),
        "w1": w1_t, "b1m": np.asarray(b1, np.float32), "w2": w2_b,
    }
    in_maps = []
    for core in range(NCORES):
        bb = core // 4
        g = core % 4
        m = dict(shared)
        m["x"] = np.ascontiguousarray(x[bb, 512 * g:512 * g + 512])
        m["maskp"] = masks[bb]
        in_maps.append(m)
    return in_maps


def kernel(**inputs):
    nc = _get_program()
    in_maps = _prep_inputs(**inputs)
    res = run_bass_kernel_spmd(nc, in_maps, core_ids=list(range(NCORES)))
    out = np.zeros((B, S, D), np.float32)
    for core in range(NCORES):
        bb = core // 4
        g = core % 4
        out[bb, 512 * g:512 * g + 512] = res.results[core]["out"]
    return out


if __name__ == "__main__":
    sys.path.insert(0, "/root/problem")
    import reference
    inputs = {k: np.asarray(v) for k, v in reference.setup_inputs().items()}
    expected = np.asarray(reference.reference(**inputs))
    actual = kernel(**inputs)
    err = np.abs(actual - expected)
    scale = np.abs(expected).max()
    print("max abs err:", err.max(), "scale:", scale, "rel:", err.max() / scale)


# revision 13
# speedup vs baseline: 1.1214x; 1.0342x over previous
"""Trainium2 Bass kernel for nn_AttentionBlock (B=2,S=2048,D=1024,H=16,MLP=4096).

Key structural insight: the reference does q.reshape(B,H,S,HD) on a row-major
[B,S,D] tensor (no transpose), so head hh consumes ROWS 128*hh:128*(hh+1) of
the projected [2048,1024] matrix reinterpreted as [2048,64]. The entire block
(LN1 -> QKV -> attention -> residual -> LN2 -> MLP -> residual) therefore
decomposes into 32 independent 128-row blocks (B*H), aligned across all ops.
Sharding: 8 cores x 4 head-blocks (512 rows) each, zero collectives.

Per-core device program (all matmuls bf16, accum f32):
  - LN1 (bn_stats) -> transpose via PE -> hT [1024,512] (per-partition affine)
  - qprojT/kprojT matmuls; rearranged into per-head [64, c, 128] blocks
    (qblk/kblk); vproj natural -> vaug with interleaved ones columns
  - scoresT per (head, e-block): 16 K=64 matmuls from qblk/kblk sub-blocks,
    + host-permuted mask add (DVE), exp (ACT, no max-subtraction: scores
    are bounded), then accumulate out^T = [V|1]^T @ expT into PSUM, which
    yields softmax numerator AND denominator in one accumulation.
  - transpose-back per 128-col block + normalize -> h_attn
  - residual + LN2 -> h2T; MLP: mlp1T = w1-chunks @ h2T, exact Gelu (ACT),
    mlp2 natural with ghT chunks as lhsT; final residual add, DMA out.
"""
import sys

sys.path.insert(0, "/opt/trn_rl_repo")

import numpy as np
import ml_dtypes

import concourse.bass as bass
import concourse.mybir as mybir
import concourse.tile as tile
from concourse.bass_utils import run_bass_kernel_spmd
from concourse.masks import make_identity

B, S, D = 2, 2048, 1024
H, HD, MLP = 16, 64, 4096
EPS = 1e-5
R = 512          # rows per core
NCORES = 8
F32 = mybir.dt.float32
BF16 = mybir.dt.bfloat16
BF = ml_dtypes.bfloat16
AF = mybir.ActivationFunctionType
AOP = mybir.AluOpType


# walrus in this container rejects >1 sync-wait on TPB_CTRL (Drain): split the
# TileContext final-drain waits across sequential drains (same AND semantics).
def _patch_drain():
    if getattr(tile.TileContext, "_dab_patched", False):
        return

    def _patched_dab(self, tick_clock, wait_clock):
        from concourse.vector_clock import ScopedClock
        drain_inst = self.nc.sync.drain()
        wait_clock.add_sem_waits(drain_inst.ins,
                                 ScopedClock({None: tick_clock.global_clock}))
        si = drain_inst.ins.sync_info
        if si is not None and len(si.on_wait) > 1:
            waits = list(si.on_wait)
            drain_inst.ins.sync_info = mybir.SyncInfo(on_wait=waits[:1],
                                                      on_update=list(si.on_update))
            for w in waits[1:]:
                extra = self.nc.sync.drain()
                extra.ins.sync_info = mybir.SyncInfo(on_wait=[w], on_update=[])
        self.nc.all_engine_barrier()
        assert self.sems is not None
        popped = self.nc._tile_sem_poison_stack.pop()
        assert popped is self._sem_poison
        self.nc.clear_and_free_semaphores(list(self.sems.allocated().values()))
        self.nc.all_engine_barrier()

    tile.TileContext._drain_and_barrier = _patched_dab
    tile.TileContext._dab_patched = True


# This walrus build accepts at most ONE sync-wait per instruction
# (setupSyncWait raises "Too many sync wait commands" otherwise).  Hoist
# excess waits onto same-engine NoOp carriers placed immediately before the
# instruction: engine streams execute in order, so waiting on the carrier
# then the instruction is equivalent to the instruction waiting on all.
_WAIT_LIMIT = 1


def _split_waits(nc):
    n_carriers = 0
    for bbname, bbw in nc.bb_map.items():
        il = bbw.bb.instructions
        out = []
        for inst in il:
            si = inst.sync_info
            if si is not None and len(si.on_wait) > _WAIT_LIMIT:
                waits = list(si.on_wait)
                extra, keep = waits[:-_WAIT_LIMIT], waits[-_WAIT_LIMIT:]
                for w in extra:
                    nop = mybir.InstNoOp(name=f"wsplit_{n_carriers}", ins=[], outs=[])
                    nop.engine = inst.engine
                    nop.sync_info = mybir.SyncInfo(on_wait=[w], on_update=[])
                    nc.register_instruction(nop, overwrite=True)
                    out.append(nop)
                    n_carriers += 1
                inst.sync_info = mybir.SyncInfo(on_wait=keep,
                                                on_update=list(si.on_update))
            out.append(inst)
        bbw.bb.instructions = out
    return n_carriers


def _phase_a(nc, tc, pp, pAB, consts, dram):
    """LN1 -> hT, QKV projections -> qblk/kblk/vaug. Returns nothing."""
    ident, eps_t, cols = consts["ident_bf"], consts["eps"], consts
    x_sb = consts["x_sb"]
    qblk, kblk, vaug, mask_sb = (consts["qblk"], consts["kblk"],
                                 consts["vaug"], consts["mask_sb"])
    with tc.tile_pool(name="phA", bufs=1) as pA, \
         tc.tile_pool(name="stat", bufs=8) as stat, \
         tc.tile_pool(name="psP", bufs=4, space="PSUM") as psP, \
         tc.tile_pool(name="wload", bufs=1) as wl, \
         tc.tile_pool(name="stg", bufs=2) as stg:
        xc = pA.tile([128, 4, D], BF16, tag="xc")
        hT = pA.tile([128, 8, R], BF16, tag="hT")
        x_r = dram["x"].rearrange("(n p) d -> n p d", p=128)
        for i in range(4):
            nc.sync.dma_start(x_sb[:, i, :], x_r[i])
            st = stat.tile([128, 2, 6], F32, tag="st")
            nc.vector.bn_stats(st[:, 0, :], x_sb[:, i, 0:512])
            nc.vector.bn_stats(st[:, 1, :], x_sb[:, i, 512:D])
            mv = stat.tile([128, 2], F32, tag="mv")
            nc.vector.bn_aggr(mv, st)
            sd = stat.tile([128, 1], F32, tag="sd")
            nc.scalar.activation(sd, mv[:, 1:2], AF.Sqrt, bias=eps_t, scale=1.0)
            rs = stat.tile([128, 1], F32, tag="rs")
            nc.vector.reciprocal(rs, sd)
            nc.vector.tensor_scalar(out=xc[:, i, :], in0=x_sb[:, i, :],
                                    scalar1=mv[:, 0:1], scalar2=rs,
                                    op0=AOP.subtract, op1=AOP.mult)
        for rt in range(4):
            nc.sync.dma_start(hT[:, :, 128 * rt:128 * rt + 128],
                              xc[:, rt, :], transpose=True)
        for dt8 in range(8):
            nc.vector.tensor_scalar(out=hT[:, dt8, :], in0=hT[:, dt8, :],
                                    scalar1=cols["g1"][:, dt8:dt8 + 1],
                                    scalar2=cols["b1"][:, dt8:dt8 + 1],
                                    op0=AOP.mult, op1=AOP.add)
        # Q and K projections (transposed layout) -> qblk/kblk
        for wname, bname, blk in (("wq", "bq", qblk), ("wk", "bk", kblk)):
            w_sb = wl.tile([128, 8, D], BF16, tag="w", name=f"w_{wname}")
            nc.sync.dma_start(w_sb, dram[wname].rearrange("(c p) d -> p c d", p=128))
            b_c = cols[bname]
            for t8 in range(8):
                ps = psP.tile([128, R], F32, tag="pp", name=f"ps_{wname}_{t8}")
                for ci in range(8):
                    nc.tensor.matmul(ps, w_sb[:, ci, 128 * t8:128 * t8 + 128],
                                     hT[:, ci, :], start=(ci == 0), stop=(ci == 7))
                sg = stg.tile([128, R], BF16, tag="sg", name=f"sg_{wname}_{t8}")
                nc.vector.tensor_scalar(out=sg, in0=ps, scalar1=b_c[:, t8:t8 + 1],
                                        scalar2=None, op0=AOP.add)
                nc.gpsimd.dma_start(blk[0:64, :, 2 * t8, :],
                                     sg[0:64, :].rearrange("p (j r) -> p j r", j=4))
                nc.gpsimd.dma_start(blk[0:64, :, 2 * t8 + 1, :],
                                    sg[64:128, :].rearrange("p (j r) -> p j r", j=4))
        # V projection (natural layout) -> vaug
        wv_sb = wl.tile([128, 8, D], BF16, tag="w")
        nc.sync.dma_start(wv_sb, dram["wv"].rearrange("(c p) d -> p c d", p=128))
        for rt in range(4):
            for hf in range(2):
                ps = psP.tile([128, R], F32, tag="pp", name=f"ps_v_{rt}_{hf}")
                for ci in range(8):
                    nc.tensor.matmul(ps, hT[:, ci, 128 * rt:128 * rt + 128],
                                     wv_sb[:, ci, 512 * hf:512 * hf + 512],
                                     start=(ci == 0), stop=(ci == 7))
                nc.vector.tensor_copy(vaug[:, rt, 8 * hf:8 * hf + 8, 0:64],
                                      ps.rearrange("p (e dd) -> p e dd", dd=64))


def _phase_b(nc, tc, consts, hattn):
    """Attention: scoresT -> mask+exp -> V_aug accumulation -> normalize."""
    ident = consts["ident"]
    qblk, kblk, vaug, mask_sb = (consts["qblk"], consts["kblk"],
                                 consts["vaug"], consts["mask_sb"])
    with tc.tile_pool(name="ex", bufs=2) as exp_pool, \
         tc.tile_pool(name="nmsb", bufs=2) as nms, \
         tc.tile_pool(name="scps", bufs=4, space="PSUM") as scps, \
         tc.tile_pool(name="outps", bufs=1, space="PSUM") as outps, \
         tc.tile_pool(name="attst", bufs=8) as attst:
        for j in range(4):
            outp = outps.tile([65, S], F32, tag="op", name=f"op_{j}")
            for p in range(8):
                e0, e1 = 2 * p, 2 * p + 1
                ex0 = exp_pool.tile([128, S], BF16, tag="ex", name=f"ex_{j}_{e0}")
                ex1 = exp_pool.tile([128, S], BF16, tag="ex", name=f"ex_{j}_{e1}")
                for q in range(4):
                    col = 512 * q
                    sc0 = scps.tile([128, 512], F32, tag="sc",
                                    name=f"sc_{j}_{e0}_{q}")
                    sc1 = scps.tile([128, 512], F32, tag="sc",
                                    name=f"sc_{j}_{e1}_{q}")
                    nc.tensor.matmul(sc0, kblk[0:64, j, e0, :],
                                     qblk[0:64, j, 4 * q:4 * q + 4, :],
                                     start=True, stop=True)
                    nc.tensor.matmul(sc1, kblk[64:128, j, e0, :],
                                     qblk[64:128, j, 4 * q:4 * q + 4, :],
                                     start=True, stop=True)
                    for sc_, ex_, e_ in ((sc0, ex0, e0), (sc1, ex1, e1)):
                        nc.scalar.activation(ex_[:, col:col + 512], sc_, AF.Exp)
                        nc.vector.tensor_tensor(
                            out=ex_[:, col:col + 512],
                            in0=ex_[:, col:col + 512],
                            in1=mask_sb[:, e_, col:col + 512],
                            op=AOP.mult)
                        nc.tensor.matmul(outp[:, col:col + 512],
                                         vaug[:, j, e_, :],
                                         ex_[:, col:col + 512],
                                         start=(e_ == 0), stop=(e_ == 15))
            nm = nms.tile([65, S], F32, tag="nm", name=f"nm_{j}")
            nc.vector.tensor_copy(nm, outp)
            for c16 in range(16):
                tp = scps.tile([128, 512], F32, tag="sc", name=f"tpb_{j}_{c16}")
                nc.tensor.transpose(tp[:, 0:65], nm[:, 128 * c16:128 * c16 + 128],
                                    ident[0:65, 0:65])
                rd = attst.tile([128, 1], F32, tag="rd", name=f"rd_{j}_{c16}")
                nc.vector.reciprocal(rd, tp[:, 64:65])
                nc.vector.tensor_scalar(out=hattn[:, j, 64 * c16:64 * c16 + 64],
                                        in0=tp[:, 0:64], scalar1=rd, scalar2=None,
                                        op0=AOP.mult)


def _phase_cd(nc, tc, consts, dram, hattn):
    """Residual + LN2 -> h2T; MLP; final residual; DMA out."""
    ident, eps_t = consts["ident"], consts["eps"]
    x_sb = consts["x_sb"]
    with tc.tile_pool(name="phC", bufs=1) as pC, \
         tc.tile_pool(name="stat2", bufs=8) as stat2, \
         tc.tile_pool(name="tmp2", bufs=2) as tmp2:
        xc2 = pC.tile([128, 4, D], F32, tag="xc2")
        h2T = pC.tile([128, 8, R], BF16, tag="h2T")
        for i in range(4):
            h2p = tmp2.tile([128, D], F32, tag="h2p", name=f"h2p_{i}")
            nc.vector.tensor_tensor(out=h2p, in0=hattn[:, i, :],
                                    in1=x_sb[:, i, :], op=AOP.add)
            nc.vector.tensor_tensor(out=h2p, in0=h2p, in1=consts["bv_b"], op=AOP.add)
            st = stat2.tile([128, 2, 6], F32, tag="st", name=f"st2_{i}")
            nc.vector.bn_stats(st[:, 0, :], h2p[:, 0:512])
            nc.vector.bn_stats(st[:, 1, :], h2p[:, 512:D])
            mv = stat2.tile([128, 2], F32, tag="mv", name=f"mv2_{i}")
            nc.vector.bn_aggr(mv, st)
            sd = stat2.tile([128, 1], F32, tag="sd", name=f"sd2_{i}")
            nc.scalar.activation(sd, mv[:, 1:2], AF.Sqrt, bias=eps_t, scale=1.0)
            rs = stat2.tile([128, 1], F32, tag="rs", name=f"rs2_{i}")
            nc.vector.reciprocal(rs, sd)
            nc.vector.tensor_scalar(out=xc2[:, i, :], in0=h2p,
                                    scalar1=mv[:, 0:1], scalar2=rs,
                                    op0=AOP.subtract, op1=AOP.mult)
        xc2b = pC.tile([128, 4, D], BF16, tag="xc2b")
        for rt in range(4):
            nc.vector.tensor_copy(xc2b[:, rt, :], xc2[:, rt, :])
        for rt in range(4):
            nc.sync.dma_start(h2T[:, :, 128 * rt:128 * rt + 128],
                              xc2b[:, rt, :], transpose=True)
        for dt8 in range(8):
            nc.vector.tensor_scalar(out=h2T[:, dt8, :], in0=h2T[:, dt8, :],
                                    scalar1=consts["g2"][:, dt8:dt8 + 1],
                                    scalar2=consts["b2l"][:, dt8:dt8 + 1],
                                    op0=AOP.mult, op1=AOP.add)
        ghT = pC.tile([128, 32, R], BF16, tag="ghT")
        with tc.tile_pool(name="w1l", bufs=2) as w1l, \
             tc.tile_pool(name="psD1", bufs=4, space="PSUM") as psD1:
            for mt4 in range(8):
                w1t = w1l.tile([128, 4, 8, 128], BF16, tag="w1", name=f"w1t_{mt4}")
                nc.scalar.dma_start(
                    w1t, dram["w1"][4 * mt4:4 * mt4 + 4].rearrange("m p c n -> p m c n"))
                for sub in range(4):
                    mt = 4 * mt4 + sub
                    ps = psD1.tile([128, R], F32, tag="m1", name=f"m1_{mt}")
                    for ci in range(8):
                        nc.tensor.matmul(ps, w1t[:, sub, ci, :], h2T[:, ci, :],
                                         start=(ci == 0), stop=(ci == 7))
                    nc.scalar.activation(ghT[:, mt, :], ps, AF.Gelu,
                                         bias=consts["b1m"][:, mt:mt + 1], scale=1.0)
        resf = pC.tile([128, 4, D], F32, tag="resf")
        for rt in range(4):
            nc.vector.tensor_tensor(out=resf[:, rt, :], in0=xc2[:, rt, :],
                                    in1=consts["g2_b"], op=AOP.mult)
            nc.vector.tensor_tensor(out=resf[:, rt, :], in0=resf[:, rt, :],
                                    in1=consts["bcomb_b"], op=AOP.add)
        ostg = pC.tile([128, 4, D], F32, tag="ostg")
        out_r = dram["out"].rearrange("(n p) d -> n p d", p=128)
        with tc.tile_pool(name="w2l", bufs=2) as w2l, \
             tc.tile_pool(name="psD2", bufs=1, space="PSUM") as psD2:
            m2ps = [[psD2.tile([128, 512], F32, tag=f"m2_{rt}_{dc}",
                               name=f"m2_{rt}_{dc}")
                     for dc in range(2)] for rt in range(4)]
            for mc4 in range(8):
                w2t = w2l.tile([128, 4, D], BF16, tag="w2", name=f"w2t_{mc4}")
                nc.scalar.dma_start(
                    w2t, dram["w2"][512 * mc4:512 * mc4 + 512, :].rearrange(
                        "(m p) d -> p m d", p=128))
                for sub in range(4):
                    mc = 4 * mc4 + sub
                    for rt in range(4):
                        for dc in range(2):
                            nc.tensor.matmul(m2ps[rt][dc],
                                             ghT[:, mc, 128 * rt:128 * rt + 128],
                                             w2t[:, sub, 512 * dc:512 * dc + 512],
                                             start=(mc == 0), stop=(mc == 31))
            for rt in range(4):
                for dc in range(2):
                    nc.vector.tensor_tensor(out=ostg[:, rt, 512 * dc:512 * dc + 512],
                                            in0=m2ps[rt][dc],
                                            in1=resf[:, rt, 512 * dc:512 * dc + 512],
                                            op=AOP.add)
                nc.sync.dma_start(out_r[rt], ostg[:, rt, :])


def _build_program():
    _patch_drain()
    nc = bass.Bass()
    dram = {
        "x": nc.dram_tensor("x", [R, D], F32, kind="ExternalInput"),
        "maskp": nc.dram_tensor("maskp", [S, S], BF16, kind="ExternalInput"),
        "wq": nc.dram_tensor("wq", [D, D], BF16, kind="ExternalInput"),
        "wk": nc.dram_tensor("wk", [D, D], BF16, kind="ExternalInput"),
        "wv": nc.dram_tensor("wv", [D, D], BF16, kind="ExternalInput"),
        "bq": nc.dram_tensor("bq", [D], F32, kind="ExternalInput"),
        "bk": nc.dram_tensor("bk", [D], F32, kind="ExternalInput"),
        "bv": nc.dram_tensor("bv", [D], F32, kind="ExternalInput"),
        "g1": nc.dram_tensor("g1", [D], F32, kind="ExternalInput"),
        "b1": nc.dram_tensor("b1", [D], F32, kind="ExternalInput"),
        "g2": nc.dram_tensor("g2", [D], F32, kind="ExternalInput"),
        "b2l": nc.dram_tensor("b2l", [D], F32, kind="ExternalInput"),
        "bcomb": nc.dram_tensor("bcomb", [D], F32, kind="ExternalInput"),
        "w1": nc.dram_tensor("w1", [32, 128, 8, 128], BF16, kind="ExternalInput"),
        "b1m": nc.dram_tensor("b1m", [MLP], F32, kind="ExternalInput"),
        "w2": nc.dram_tensor("w2", [MLP, D], BF16, kind="ExternalInput"),
        "out": nc.dram_tensor("out", [R, D], F32, kind="ExternalOutput"),
    }
    with tile.TileContext(nc) as tc:
        with tc.tile_pool(name="persist", bufs=1) as pp:
            consts = {}
            ident = pp.tile([128, 128], F32, tag="ident")
            make_identity(nc, ident)
            consts["ident"] = ident
            ident_bf = pp.tile([128, 128], BF16, tag="identbf")
            make_identity(nc, ident_bf)
            consts["ident_bf"] = ident_bf
            eps_t = pp.tile([128, 1], F32, tag="eps")
            nc.vector.memset(eps_t, EPS)
            consts["eps"] = eps_t

            def pcol(key, n):
                t_ = pp.tile([128, n], F32, tag=f"c_{key}", name=f"c_{key}")
                nc.sync.dma_start(t_, dram[key].rearrange("(t p) -> p t", p=128))
                return t_

            for key, n in (("bq", 8), ("bk", 8), ("g1", 8), ("b1", 8),
                           ("g2", 8), ("b2l", 8), ("b1m", 32)):
                consts[key] = pcol(key, n)

            def bcast(key):
                t_ = pp.tile([128, D], F32, tag=f"b_{key}", name=f"b_{key}")
                src = dram[key][:]
                bc = bass.AP(tensor=src.tensor, offset=src.offset,
                             ap=[[0, 128]] + [list(a) for a in src.ap])
                nc.sync.dma_start(t_, bc)
                return t_

            consts["bv_b"] = bcast("bv")
            consts["g2_b"] = bcast("g2")
            consts["bcomb_b"] = bcast("bcomb")

            consts["x_sb"] = pp.tile([128, 4, D], F32, tag="x", name="x_sb")

            with tc.tile_pool(name="poolBC", bufs=1) as pBC:
                with tc.tile_pool(name="poolAB", bufs=1) as pAB:
                    consts["qblk"] = pAB.tile([128, 4, 16, 128], BF16, tag="qblk",
                                              name="qblk")   # [d, j, c, rr]; hi=dup
                    consts["kblk"] = pAB.tile([128, 4, 16, 128], BF16, tag="kblk",
                                              name="kblk")   # hi = e+1 shift
                    consts["vaug"] = pAB.tile([128, 4, 16, 65], BF16, tag="vaug",
                                              name="vaug")   # [rr, j, e, d|1]
                    nc.vector.memset(consts["vaug"][:, :, :, 64:65], 1.0)
                    consts["mask_sb"] = pAB.tile([128, 16, S], BF16, tag="mask",
                                                 name="mask_sb")
                    _phase_a(nc, tc, pp, pAB, consts, dram)
                    nc.sync.dma_start(consts["qblk"][64:128, :, :, :],
                                      consts["qblk"][0:64, :, :, :])
                    nc.sync.dma_start(consts["kblk"][64:128, :, 0:15, :],
                                      consts["kblk"][0:64, :, 1:16, :])
                    mask_r = dram["maskp"].rearrange("(g e p) s -> g p e s", p=128, e=4)
                    for g in range(4):
                        nc.scalar.dma_start(consts["mask_sb"][:, 4 * g:4 * g + 4, :],
                                            mask_r[g])
                    hattn = pBC.tile([128, 4, D], F32, tag="hattn", name="hattn")
                    _phase_b(nc, tc, consts, hattn)
                _phase_cd(nc, tc, consts, dram, hattn)
    n = _split_waits(nc)
    print(f"[kernel] split {n} excess sync-waits onto NoOp carriers")
    return nc


_PROGRAM = None


def _get_program():
    global _PROGRAM
    if _PROGRAM is None:
        _PROGRAM = _build_program()
    return _PROGRAM


def _prep_inputs(x, dis_attn_mask, cls_attn_mask, wq, bq, wk, bk, wv, bv,
                 ln1_g, ln1_b, ln2_g, ln2_b, w1, b1, w2, b2):
    """Host-side prep: per-core shards + weight dtype/layout conversion."""
    x = np.asarray(x, np.float32)
    wq8 = (np.asarray(wq, np.float32) / 8.0).astype(BF)
    bq8 = (np.asarray(bq, np.float32) / 8.0)
    wk_b = np.asarray(wk, np.float32).astype(BF)
    wv_b = np.asarray(wv, np.float32).astype(BF)
    # w1 pre-tiled to [32(mt), 128(p=din%128), 8(c=din//128), 128(n=mout%128)]
    w1_t = np.ascontiguousarray(
        np.asarray(w1, np.float32).astype(BF).reshape(8, 128, 32, 128).transpose(2, 1, 0, 3))
    w2_b = np.asarray(w2, np.float32).astype(BF)
    bcomb = (np.asarray(ln2_b, np.float32) + np.asarray(b2, np.float32))
    masks = {}
    for bb in range(B):
        msum = (np.asarray(dis_attn_mask[bb], np.float32)
                + np.asarray(cls_attn_mask[bb], np.float32))
        masks[bb] = np.exp(np.ascontiguousarray(
            msum.reshape(128, 16, 128, 16).transpose(3, 2, 1, 0).reshape(S, S))).astype(BF)
    shared = {
        "wq": wq8, "wk": wk_b, "wv": wv_b,
        "bq": bq8.astype(np.float32), "bk": np.asarray(bk, np.float32),
        "bv": np.asarray(bv, np.float32),
        "g1": np.asarray(ln1_g, np.float32), "b1": np.asarray(ln1_b, np.float32),
        "g2": np.asarray(ln2_g, np.float32), "b2l": np.asarray(ln2_b, np.float32),
        "bcomb": bcomb.astype(np.float32),
        "w1": w1_t, "b1m": np.asarray(b1, np.float32), "w2": w2_b,
    }
    in_maps = []
    for core in range(NCORES):
        bb = core // 4
        g = core % 4
        m = dict(shared)
        m["x"] = np.ascontiguousarray(x[bb, 512 * g:512 * g + 512])
        m["maskp"] = masks[bb]
        in_maps.append(m)
    return in_maps


def kernel(**inputs):
    nc = _get_program()
    in_maps = _prep_inputs(**inputs)
    res = run_bass_kernel_spmd(nc, in_maps, core_ids=list(range(NCORES)))
    out = np.zeros((B, S, D), np.float32)
    for core in range(NCORES):
        bb = core // 4
        g = core % 4
        out[bb, 512 * g:512 * g + 512] = res.results[core]["out"]
    return out


if __name__ == "__main__":
    sys.path.insert(0, "/root/problem")
    import reference
    inputs = {k: np.asarray(v) for k, v in reference.setup_inputs().items()}
    expected = np.asarray(reference.reference(**inputs))
    actual = kernel(**inputs)
    err = np.abs(actual - expected)
    scale = np.abs(expected).max()
    print("max abs err:", err.max(), "scale:", scale, "rel:", err.max() / scale)
